# revision 1
# baseline (speedup 1.0000x reference)
"""Causal single-head attention (B=4, T=4096, C=512, D=64) on 8 TRN2 NeuronCores.

Sharding: core c -> (batch b = c // 2, parity P = c % 2).  Each batch's 32
q-tiles (128 rows each) are striped by parity: core (b, P) owns global q-tiles
k = 2j + P, j = 0..15.  Slot j's causal kv extent is padded to 256*(j+1) keys
(uniform across parities, +3% work) and the last 256 key columns get a
parity-specific additive mask fed as input data, so one SPMD program serves
all 8 cores.

Per-core dataflow:
  phase 1: K^T|V^T from a single W-stationary matmul per x^T chunk (Wk and Wv
           stacked into one 128-wide stationary operand; C=512 contracted in
           4 chunks of 128); V^T is PE-transposed into fp32 V[S, D+1] tiles
           whose last column is 1.0 (gives row sums for free during AV).
           Q^T/K^T are mirrored into the upper 64 SBUF partitions so the
           scores matmuls run 2x row-tiled (K=64 uses half the PE array; two
           concurrent 64-row tiles).  K-hat = [K^T; ones] and Q-hat =
           [Q^T; -m_row] buffers serve the transposed-scores matmul.
  phase 2 (flash, per slot j, groups of up to 1536 keys):
           1. scores S = Q_j K^T on PE (row-tiled, 512-wide PSUM chunks),
              additive -1e30 mask on the slot's last 256 columns, DVE row-max
              -> running max m (these scores are used ONLY for the max).
           2. -m written as a [1,128] row into Q-hat partition 64 (PE
              transpose of m via identity matmul + DVE negate-copy).
           3. S^T - m computed directly on PE via the 65-deep contraction
              [K^T; 1]^T [Q^T; -m] -> one ACT exp (scale=8) writes A^T
              straight to SBUF (no PSUM->SBUF copy pass, no PE transposes
              of A), masked via a transposed mask on the last two blocks.
           4. AV: po[128, 65] += A^T_block^T V-hat_block on PE; column 64
              accumulates the row sums l.  Running rescale of (O, l) by
              exp(8*(m_old - m_new)) on DVE; final y = O / l.
"""

import numpy as np

B, T, C, D = 4, 4096, 512, 64
P128 = 128
NSLOT = 16          # q-tile slots per core
TQ = NSLOT * P128   # 2048 q rows per core
NEG = -1.0e30
GRP = 1536

_CACHED = {}


def _build(use_dma_t=False, rowtile=True, st_mode=True, grp=GRP):
    import concourse.bass as bass
    import concourse.mybir as mybir
    from concourse import bacc
    from concourse.tile import TileContext
    from concourse.masks import make_identity

    f32 = mybir.dt.float32
    bf16 = mybir.dt.bfloat16
    AX = mybir.AxisListType.X
    ALU = mybir.AluOpType
    ACTF = mybir.ActivationFunctionType

    nc = bacc.Bacc("TRN2", target_bir_lowering=False, debug=False,
                   enable_asserts=False, num_devices=8)

    xT = nc.dram_tensor("xT", [C, T], f32, kind="ExternalInput").ap()
    xTq = nc.dram_tensor("xTq", [C, TQ], f32, kind="ExternalInput").ap()
    wq = nc.dram_tensor("wq", [C, D], f32, kind="ExternalInput").ap()
    wk = nc.dram_tensor("wk", [C, D], f32, kind="ExternalInput").ap()
    wv = nc.dram_tensor("wv", [C, D], f32, kind="ExternalInput").ap()
    bq = nc.dram_tensor("bq", [D, 1], f32, kind="ExternalInput").ap()
    bk = nc.dram_tensor("bk", [D, 1], f32, kind="ExternalInput").ap()
    bv = nc.dram_tensor("bv", [D, 1], f32, kind="ExternalInput").ap()
    maskp = nc.dram_tensor("maskp", [P128, 512], f32, kind="ExternalInput").ap()
    maskpT = nc.dram_tensor("maskpT", [P128, 256], f32, kind="ExternalInput").ap()
    y = nc.dram_tensor("y", [TQ, D], f32, kind="ExternalOutput").ap()
    DV = D + 1 if st_mode else D  # V tiles carry a ones column in st_mode

    with TileContext(nc) as tc:
        with (
            tc.tile_pool(name="singles", bufs=1) as singles,
            tc.tile_pool(name="xin", bufs=3) as xin,
            tc.tile_pool(name="work", bufs=2) as work,
            tc.tile_pool(name="small", bufs=3) as small,
            tc.tile_pool(name="ps_s", bufs=2, space="PSUM") as ps_s,
            tc.tile_pool(name="ps_s2", bufs=1, space="PSUM") as ps_s2,
            tc.tile_pool(name="ps_t", bufs=1, space="PSUM") as ps_t,
            tc.tile_pool(name="ps_o", bufs=1, space="PSUM") as ps_o,
            tc.tile_pool(name="ps_p", bufs=1, space="PSUM") as ps_p,
        ):
            # ---- resident constants (SWDGE loads) ----
            wqs = singles.tile([P128, 4, D], f32, tag="wqs")
            wkv = singles.tile([P128, 4, 2 * D], f32, tag="wkv")
            nc.gpsimd.dma_start(out=wqs, in_=wq.rearrange("(c p) d -> p c d", p=P128))
            nc.gpsimd.dma_start(out=wkv[:, :, :D],
                                in_=wk.rearrange("(c p) d -> p c d", p=P128))
            nc.gpsimd.dma_start(out=wkv[:, :, D:],
                                in_=wv.rearrange("(c p) d -> p c d", p=P128))
            bqs = singles.tile([D, 1], f32, tag="bqs")
            bks = singles.tile([D, 1], f32, tag="bks")
            bvs = singles.tile([D, 1], f32, tag="bvs")
            nc.gpsimd.dma_start(out=bqs, in_=bq)
            nc.gpsimd.dma_start(out=bks, in_=bk)
            nc.gpsimd.dma_start(out=bvs, in_=bv)
            msk = singles.tile([P128, 512], f32, tag="msk")
            nc.gpsimd.dma_start(out=msk, in_=maskp)
            if st_mode:
                mskT = singles.tile([P128, 256], f32, tag="mskT")
                nc.gpsimd.dma_start(out=mskT, in_=maskpT)
                identf = singles.tile([P128, P128], f32, tag="identf")
                make_identity(nc, identf)
            else:
                mskT = identf = None

            QP = P128 if rowtile else D
            QTo = singles.tile([QP, TQ], f32, tag="QTo")
            KT = singles.tile([QP, T], f32, tag="KT")
            Vsb = singles.tile([P128, (T // P128) * DV], f32, tag="Vsb")
            if st_mode:
                # K-hat: [K^T; ones] and Q-hat: [Q^T; -m_row] for the shifted
                # transposed-scores matmul (rank-1 max subtraction in-contraction)
                KH = singles.tile([D + 1, T], f32, tag="KH")
                QH = singles.tile([D + 1, TQ], f32, tag="QH")
                nc.vector.memset(KH[D:D + 1, :], 1.0)
                nc.vector.memset(Vsb, 1.0)   # ones column survives V writes

            # ---- phase 1: projections ----
            for t8 in range(T // 512):
                xt = xin.tile([P128, 4, 512], f32, tag="xt")
                nc.gpsimd.dma_start(
                    out=xt,
                    in_=xT[:, t8 * 512:(t8 + 1) * 512].rearrange(
                        "(c p) n -> p c n", p=P128),
                )
                kvps = ps_p.tile([2 * D, 512], f32, tag="pp")
                for c in range(4):
                    nc.tensor.matmul(kvps, wkv[:, c, :], xt[:, c, :],
                                     start=(c == 0), stop=(c == 3))
                nc.scalar.activation(KT[:D, t8 * 512:(t8 + 1) * 512], kvps[:D, :],
                                     ACTF.Identity, bias=bks, scale=1.0)
                vtmp = work.tile([D, 512], f32, tag="vtmp")
                nc.scalar.activation(vtmp, kvps[D:, :], ACTF.Identity,
                                     bias=bvs, scale=1.0)
                # transpose V^T [64, 128] blocks -> bf16 V [128, 64] tiles
                for i in range(4):
                    t = t8 * 4 + i
                    if use_dma_t:
                        nc.sync.dma_start(
                            out=Vsb[:, t * DV:t * DV + D],
                            in_=vtmp[:, i * P128:(i + 1) * P128], transpose=True)
                    else:
                        pt = ps_t.tile([P128, 512], f32, tag="pt")
                        nc.tensor.transpose(pt[:, :D],
                                            vtmp[:, i * P128:(i + 1) * P128],
                                            identf[:D, :D])
                        nc.vector.tensor_copy(Vsb[:, t * DV:t * DV + D], pt[:, :D])
            # Q^T from xTq (own 2048 rows)
            for t8 in range(TQ // 512):
                xt = xin.tile([P128, 4, 512], f32, tag="xt")
                nc.gpsimd.dma_start(
                    out=xt,
                    in_=xTq[:, t8 * 512:(t8 + 1) * 512].rearrange(
                        "(c p) n -> p c n", p=P128),
                )
                qps = ps_p.tile([D, 512], f32, tag="pp")
                for c in range(4):
                    nc.tensor.matmul(qps, wqs[:, c, :], xt[:, c, :],
                                     start=(c == 0), stop=(c == 3))
                nc.scalar.activation(QTo[:D, t8 * 512:(t8 + 1) * 512], qps,
                                     ACTF.Identity, bias=bqs, scale=1.0)
            if rowtile:
                # mirror Q^T/K^T into the upper 64 partitions for row tiling
                nc.gpsimd.dma_start(out=KT[D:2 * D, :], in_=KT[:D, :])
                nc.gpsimd.dma_start(out=QTo[D:2 * D, :], in_=QTo[:D, :])
            if st_mode:
                nc.gpsimd.dma_start(out=KH[:D, :], in_=KT[:D, :])
                nc.gpsimd.dma_start(out=QH[:D, :], in_=QTo[:D, :])

            # ---- phase 2: per-slot flash attention ----
            for j in range(NSLOT):
                ncols = 256 * (j + 1)
                groups = []
                off = 0
                while off < ncols:
                    groups.append((off, min(grp, ncols - off)))
                    off += grp

                mrun = small.tile([P128, 1], f32, tag="mrun")
                lrun = small.tile([P128, 1], f32, tag="lrun")
                Oacc = small.tile([P128, D], f32, tag="Oacc")

                for gi, (off, w) in enumerate(groups):
                    last = (gi == len(groups) - 1)
                    subs = list(range(0, w, 512))
                    mgp = small.tile([P128, 4], f32, tag="mgp")
                    for si, soff in enumerate(subs):
                        sw = min(512, w - soff)
                        half = ((off + soff) // 512) % 2 if rowtile else 0
                        pbase = half * D
                        ps = ps_s.tile([P128, 512], f32, tag="ps")
                        nc.tensor.matmul(
                            ps[:, :sw],
                            QTo[pbase:pbase + D, j * P128:(j + 1) * P128],
                            KT[pbase:pbase + D, off + soff:off + soff + sw],
                            start=True, stop=True)
                        if last and si == len(subs) - 1:
                            nc.vector.tensor_add(ps[:, sw - 256:sw],
                                                 ps[:, sw - 256:sw],
                                                 msk[:, 256:512])
                        nc.vector.reduce_max(mgp[:, si:si + 1], ps[:, :sw], axis=AX)
                    mg = small.tile([P128, 1], f32, tag="mg")
                    if len(subs) > 1:
                        nc.vector.reduce_max(mg, mgp[:, :len(subs)], axis=AX)
                    else:
                        nc.vector.tensor_copy(mg, mgp[:, :1])
                    if gi == 0:
                        nc.vector.tensor_copy(mrun, mg)
                    else:
                        mnew = small.tile([P128, 1], f32, tag="mnew")
                        nc.vector.tensor_max(mnew, mrun, mg)
                        mdiff = small.tile([P128, 1], f32, tag="mdiff")
                        nc.vector.tensor_sub(mdiff, mrun, mnew)
                        cstep = small.tile([P128, 1], f32, tag="cstep")
                        nc.scalar.activation(cstep, mdiff, ACTF.Exp,
                                             bias=0.0, scale=8.0)
                        nc.vector.tensor_copy(mrun, mnew)
                    nblk = w // P128
                    base = off // P128
                    AT = work.tile([P128, grp], f32, tag="AT")
                    po = ps_o.tile([P128, DV], f32, tag="po")
                    if st_mode:
                        # -m_new as a [1, 128] row at QH partition 64 (via PE)
                        pm = ps_t.tile([P128, P128], f32, tag="pt")
                        nc.tensor.matmul(pm[:1, :P128], mrun, identf,
                                         start=True, stop=True)
                        nc.vector.tensor_scalar_mul(
                            QH[D:D + 1, j * P128:(j + 1) * P128],
                            pm[:1, :P128], -1.0)
                        # shifted transposed scores: S^T - m  (65-contraction)
                        ps2 = ps_s2.tile([P128, grp], f32, tag="ps2")
                        for i in range(nblk):
                            nc.tensor.matmul(
                                ps2[:, i * P128:(i + 1) * P128],
                                KH[:, off + i * P128:off + (i + 1) * P128],
                                QH[:, j * P128:(j + 1) * P128],
                                start=True, stop=True)
                        if last:
                            nc.vector.tensor_add(
                                ps2[:, w - 256:w - P128], ps2[:, w - 256:w - P128],
                                mskT[:, 0:P128])
                            nc.vector.tensor_add(
                                ps2[:, w - P128:w], ps2[:, w - P128:w],
                                mskT[:, P128:256])
                        nc.scalar.activation(AT[:, :w], ps2[:, :w], ACTF.Exp,
                                             bias=0.0, scale=8.0)
                    else:
                        raise NotImplementedError("non-st_mode path removed")
                    for i in range(nblk):
                        nc.tensor.matmul(po, AT[:, i * P128:(i + 1) * P128],
                                         Vsb[:, (base + i) * DV:(base + i) * DV + DV],
                                         start=(i == 0), stop=(i == nblk - 1))

                    lg_ap = po[:, D:D + 1]
                    if gi == 0:
                        nc.vector.tensor_copy(Oacc, po[:, :D])
                        nc.vector.tensor_copy(lrun, lg_ap)
                    else:
                        nc.vector.scalar_tensor_tensor(
                            out=Oacc, in0=Oacc, scalar=cstep, in1=po[:, :D],
                            op0=ALU.mult, op1=ALU.add)
                        nc.vector.scalar_tensor_tensor(
                            out=lrun, in0=lrun, scalar=cstep, in1=lg_ap,
                            op0=ALU.mult, op1=ALU.add)

                rl = small.tile([P128, 1], f32, tag="rl")
                nc.vector.reciprocal(rl, lrun)
                yt = small.tile([P128, D], f32, tag="yt")
                nc.vector.tensor_scalar_mul(yt, Oacc, rl)
                nc.gpsimd.dma_start(out=y[j * P128:(j + 1) * P128, :], in_=yt)

    nc.compile()
    return nc


def _get_nc():
    if "nc" not in _CACHED:
        _CACHED["nc"] = _build()
    return _CACHED["nc"]


def _prep_in_maps(x, Wq, bq, Wk, bk, Wv, bv):
    x = np.asarray(x, dtype=np.float32)
    Wq = np.asarray(Wq, dtype=np.float32)
    Wk = np.asarray(Wk, dtype=np.float32)
    Wv = np.asarray(Wv, dtype=np.float32)
    bq_ = np.asarray(bq, dtype=np.float32).reshape(D, 1)
    bk_ = np.asarray(bk, dtype=np.float32).reshape(D, 1)
    bv_ = np.asarray(bv, dtype=np.float32).reshape(D, 1)

    tri = np.triu(np.ones((P128, P128), np.float32), k=1) * NEG
    masks = []
    for P in range(2):
        mp = np.zeros((P128, 512), np.float32)
        if P == 0:
            mp[:, 256:384] = tri
            mp[:, 384:512] = NEG
        else:
            mp[:, 384:512] = tri
        masks.append(mp)

    masksT = []
    for P in range(2):
        mt = np.zeros((P128, 256), np.float32)
        mt[:, 0:128] = masks[P][:, 256:384].T
        mt[:, 128:256] = masks[P][:, 384:512].T
        masksT.append(mt)

    in_maps = []
    for c in range(8):
        b, P = c // 2, c % 2
        xb = x[b]                                   # [T, C]
        rows = (np.arange(NSLOT) * 2 + P)[:, None] * P128 + np.arange(P128)[None, :]
        rows = rows.reshape(-1)
        in_maps.append({
            "xT": np.ascontiguousarray(xb.T),
            "xTq": np.ascontiguousarray(xb[rows].T),
            "wq": Wq, "wk": Wk, "wv": Wv,
            "bq": bq_, "bk": bk_, "bv": bv_,
            "maskp": masks[P], "maskpT": masksT[P],
        })
    return in_maps


def _unshard(res):
    out = np.empty((B, T, D), np.float32)
    for c in range(8):
        b, P = c // 2, c % 2
        yl = res.results[c]["y"]
        for j in range(NSLOT):
            k = 2 * j + P
            out[b, k * P128:(k + 1) * P128] = yl[j * P128:(j + 1) * P128]
    return out


def kernel(x, Wq, bq, Wk, bk, Wv, bv):
    from concourse.bass_utils import run_bass_kernel_spmd

    in_maps = _prep_in_maps(x, Wq, bq, Wk, bk, Wv, bv)
    res = run_bass_kernel_spmd(_get_nc(), in_maps, core_ids=list(range(8)))
    _CACHED["last_results"] = res
    return _unshard(res)


def run_profiled(np_inputs):
    from concourse.bass_utils import run_bass_kernel_spmd

    in_maps = _prep_in_maps(**np_inputs)
    res = run_bass_kernel_spmd(_get_nc(), in_maps, core_ids=list(range(8)),
                               trace=True)
    _CACHED["last_results"] = res
    return res


if __name__ == "__main__":
    rng = np.random.default_rng(0)
    x = rng.standard_normal((B, T, C), dtype=np.float32)
    s = 1.0 / np.sqrt(C)
    Wq = rng.standard_normal((C, D), dtype=np.float32) * s
    Wk = rng.standard_normal((C, D), dtype=np.float32) * s
    Wv = rng.standard_normal((C, D), dtype=np.float32) * s
    z = np.zeros(D, np.float32)
    print(kernel(x, Wq, z, Wk, z, Wv, z).shape)



# revision 37
# speedup vs baseline: 1.9645x; 1.9645x over previous
"""Causal single-head attention (B=4, T=4096, C=512, D=64) on 8 TRN2 NeuronCores.

Sharding: core c -> (batch b = c // 2, parity P = c % 2).  Each batch's 32
q-tiles (128 rows each) are striped by parity: core (b, P) owns global q-tiles
k = 2j + P, j = 0..15.  Slot j's causal kv extent is padded to 256*(j+1) keys
(uniform across parities) and the last 256 key columns get a parity-specific
additive mask fed as input data, so one SPMD program serves all 8 cores.

Precision plan (calibrated on HW probes): projections run fp32 on PE (score
logits are x8-scaled, sigma ~64 -- tf32-rounded projections alone breach the
2e-2 gate).  Q-hat/K-hat are stored float32r (tf32, one rounding); both score
passes run f32r at 1 cycle/col (pass-2 slot-paired so the moving dim is 256).
A and V are bf16 (rounding after exp / on values is harmless).

Per-core dataflow, slots processed in pairs (a=2i, b=2i+1):
  stream per i: project K/V chunk i (keys 512i..512i+512), project Q pair i,
  then flash pair i (its kv extent 512(i+1) is now resident):
  1. pass-1 per slot: S = Q_s K^T in f32r, 512-wide PSUM chunks; row-max on
     DVE, pairs of chunks fused via tensor_tensor_reduce(max, max), causal
     mask folded into the last chunk via tensor_tensor_reduce(add, max);
     final negated max -> -m (fp16) -> [1,128] row via PE identity transpose
     -> QH partition 64 (f32r).
  2. pass-2 per pair: S^T - m via the 65-deep contraction [K^T;1]^T[Q^T;-m],
     moving = 256 q-cols (both slots) per key block; b-only tail blocks
     (last 512 keys) moving 128.  Transposed causal masks added on PSUM
     (DVE), one ACT exp (scale=8) per 1024-wide group writes A^T bf16.
  3. AV per slot: po[128, 65] += A^T_block^T V-hat_block (bf16); column 64
     accumulates l via V-hat's ones column; y = O / l on DVE.
"""

import numpy as np

B, T, C, D = 4, 4096, 512, 64
P128 = 128
NSLOT = 16          # q-tile slots per core
NPAIR = NSLOT // 2
TQ = NSLOT * P128   # 2048 q rows per core
NEG = -1.0e30

_CACHED = {}


def _build():
    import concourse.mybir as mybir
    from concourse import bacc
    from concourse.tile import TileContext
    from concourse.masks import make_identity

    f32 = mybir.dt.float32
    f32r = mybir.dt.float32r
    f16 = mybir.dt.float16
    bf16 = mybir.dt.bfloat16
    AX = mybir.AxisListType.X
    ALU = mybir.AluOpType
    ACTF = mybir.ActivationFunctionType

    nc = bacc.Bacc("TRN2", target_bir_lowering=False, debug=False,
                   enable_asserts=False, num_devices=8)

    xT = nc.dram_tensor("xT", [2 * C, T], f16, kind="ExternalInput").ap()
    xTq = nc.dram_tensor("xTq", [2 * C, TQ], f16, kind="ExternalInput").ap()
    wqh = nc.dram_tensor("wqh", [2 * C, D], f16, kind="ExternalInput").ap()
    wkv2 = nc.dram_tensor("wkv2", [2 * C, 2 * D], f16, kind="ExternalInput").ap()
    bq = nc.dram_tensor("bq", [D, 1], f32, kind="ExternalInput").ap()
    bk = nc.dram_tensor("bk", [D, 1], f32, kind="ExternalInput").ap()
    bv = nc.dram_tensor("bv", [D, 1], f32, kind="ExternalInput").ap()
    maskp = nc.dram_tensor("maskp", [P128, 512], f32, kind="ExternalInput").ap()
    maskpT = nc.dram_tensor("maskpT", [P128, 256], f32, kind="ExternalInput").ap()
    onesr = nc.dram_tensor("onesr", [1, T], f32r, kind="ExternalInput").ap()
    y = nc.dram_tensor("y", [TQ, D], f32, kind="ExternalOutput").ap()
    DV = D + 1  # V tiles carry a ones column

    with TileContext(nc) as tc:
        with (
            tc.tile_pool(name="singles", bufs=1) as singles,
            tc.tile_pool(name="xin", bufs=3) as xin,
            tc.tile_pool(name="qin", bufs=2) as qin,
            tc.tile_pool(name="atp", bufs=2) as atp,
            tc.tile_pool(name="vwork", bufs=2) as vwork,
            tc.tile_pool(name="small", bufs=4) as small,
            tc.tile_pool(name="ps_s", bufs=3, space="PSUM") as ps_s,
            tc.tile_pool(name="ps_e", bufs=3, space="PSUM") as ps_e,
            tc.tile_pool(name="ps_o", bufs=1, space="PSUM") as ps_o,
            tc.tile_pool(name="ps_m", bufs=1, space="PSUM") as ps_m,
        ):
            # ---- resident constants ----
            wqs = singles.tile([P128, 8, D], f16, tag="wqs")
            wkv = singles.tile([P128, 8, 2 * D], f16, tag="wkv")
            nc.gpsimd.dma_start(out=wqs, in_=wqh.rearrange("(c p) d -> p c d", p=P128))
            nc.gpsimd.dma_start(out=wkv, in_=wkv2.rearrange("(c p) d -> p c d", p=P128))
            bqs = singles.tile([D, 1], f32, tag="bqs")
            bks = singles.tile([D, 1], f32, tag="bks")
            bvs = singles.tile([D, 1], f32, tag="bvs")
            nc.gpsimd.dma_start(out=bqs, in_=bq)
            nc.gpsimd.dma_start(out=bks, in_=bk)
            nc.gpsimd.dma_start(out=bvs, in_=bv)
            msk = singles.tile([P128, 512], f32, tag="msk")
            nc.gpsimd.dma_start(out=msk, in_=maskp)
            mskT = singles.tile([P128, 256], f32, tag="mskT")
            nc.gpsimd.dma_start(out=mskT, in_=maskpT)
            identb = singles.tile([P128, P128], bf16, tag="identb")
            make_identity(nc, identb)
            identh = singles.tile([P128, P128], f16, tag="identh")
            make_identity(nc, identh)

            # K-hat [K^T; ones] / Q-hat [Q^T; -m] in f32r (tf32-rounded once)
            KH = singles.tile([D + 1, T], f32r, tag="KH")
            QH = singles.tile([D + 1, TQ], f32r, tag="QH")
            nc.gpsimd.dma_start(out=KH[D:D + 1, :], in_=onesr)
            Vsb = singles.tile([P128, (T // P128) * DV], bf16, tag="Vsb")
            nc.vector.memset(Vsb, 1.0)   # ones column survives V writes
            scr = singles.tile([1, 1], f32, tag="scr")
            nc.vector.memset(scr, 0.0)
            nc.scalar.activation(scr, scr, ACTF.Exp, bias=0.0, scale=1.0)

            xts, xqs = {}, {}

            def load_kv(t8):
                xts[t8] = xin.tile([P128, 8, 512], f16, tag="xt", name=f"xt{t8}")
                nc.sync.dma_start(
                    out=xts[t8],
                    in_=xT[:, t8 * 512:(t8 + 1) * 512].rearrange(
                        "(c p) n -> p c n", p=P128))

            def load_q(i):
                # pair i's q columns: [256*i, 256*i+256)
                xqs[i] = qin.tile([P128, 8, 256], f16, tag="xq", name=f"xq{i}")
                nc.sync.dma_start(
                    out=xqs[i],
                    in_=xTq[:, i * 256:(i + 1) * 256].rearrange(
                        "(c p) n -> p c n", p=P128))

            def proj_kv(t8):
                xt = xts.pop(t8)
                kvps = ps_s.tile([P128, 512], f32, tag="ps")
                pairs = ([(c, c) for c in range(4)]
                         + [(c, c + 4) for c in range(4)]
                         + [(c + 4, c) for c in range(4)])
                for n, (wc, xc) in enumerate(pairs):
                    nc.tensor.matmul(kvps, wkv[:, wc, :], xt[:, xc, :],
                                     start=(n == 0), stop=(n == len(pairs) - 1))
                nc.scalar.activation(KH[:D, t8 * 512:(t8 + 1) * 512],
                                     kvps[:D, :], ACTF.Identity,
                                     bias=bks, scale=1.0)
                vtmp = vwork.tile([D, 512], bf16, tag="vtmp")
                nc.scalar.activation(vtmp, kvps[D:, :], ACTF.Identity,
                                     bias=bvs, scale=1.0)
                # transpose V^T [64, 128] blocks -> bf16 V-hat [128, 65] tiles
                for i in range(4):
                    t = t8 * 4 + i
                    pt = ps_m.tile([P128, 512], bf16, tag="pm")
                    nc.tensor.transpose(pt[:, :D],
                                        vtmp[:, i * P128:(i + 1) * P128],
                                        identb[:D, :D])
                    nc.vector.tensor_copy(Vsb[:, t * DV:t * DV + D], pt[:, :D])

            def proj_q(i):
                # pair i's Q: 256 columns
                xt = xqs.pop(i)
                qps = ps_s.tile([P128, 512], f32, tag="ps")
                pairs = ([(c, c) for c in range(4)]
                         + [(c, c + 4) for c in range(4)]
                         + [(c + 4, c) for c in range(4)])
                for n, (wc, xc) in enumerate(pairs):
                    nc.tensor.matmul(qps[:D, :256], wqs[:, wc, :], xt[:, xc, :],
                                     start=(n == 0), stop=(n == len(pairs) - 1))
                nc.scalar.activation(QH[:D, i * 256:(i + 1) * 256],
                                     qps[:D, :256], ACTF.Identity,
                                     bias=bqs, scale=1.0)

            def gen_pass1_pair(i):
                """Row max per slot of pair i; writes -m into QH.  Yields
                between emission units so pass-2 of the previous pair can be
                interleaved into the in-order PE queue (keeps PE busy while
                DVE drains pass-1 PSUM chunks; preserves the p-state ramp)."""
                for s in (2 * i, 2 * i + 1):
                    ncols = 256 * (s + 1)
                    chunks = [(off, min(512, ncols - off))
                              for off in range(0, ncols, 512)]
                    mgp = small.tile([P128, 8], f32, tag="mgp")
                    ci = 0
                    qsl = QH[:D, s * P128:(s + 1) * P128]
                    for (off, sw) in chunks[:-1]:
                        ps = ps_s.tile([P128, 512], f32, tag="ps")
                        nc.tensor.matmul(ps, qsl, KH[:D, off:off + sw],
                                         start=True, stop=True)
                        nc.vector.reduce_max(mgp[:, ci:ci + 1], ps, axis=AX)
                        ci += 1
                        yield
                    off, sw = chunks[-1]
                    ps = ps_s.tile([P128, 512], f32, tag="ps")
                    nc.tensor.matmul(ps[:, :sw], qsl, KH[:D, off:off + sw],
                                     start=True, stop=True)
                    nc.vector.tensor_add(ps[:, :sw], ps[:, :sw],
                                         msk[:, 512 - sw:])
                    nc.vector.reduce_max(mgp[:, ci:ci + 1], ps[:, :sw],
                                         axis=AX)
                    ci += 1
                    yield
                    mrunh = small.tile([P128, 1], bf16, tag="mrunh")
                    nc.vector.reduce_max(mrunh, mgp[:, :ci], axis=AX,
                                         negate=True)
                    # -m -> [1,128] row via PE transpose -> QH partition 64
                    pm = ps_m.tile([P128, 512], bf16, tag="pm")
                    nc.tensor.transpose(pm[:1, :P128], mrunh, identb)
                    nc.vector.tensor_copy(
                        QH[D:D + 1, s * P128:(s + 1) * P128], pm[:1, :P128])
                    yield

            def gen_pass2_pair(i):
                a, b = 2 * i, 2 * i + 1
                nfull = 4 * i + 2          # key blocks where both slots attend
                ecols = nfull * 256 + 256  # pass-2 staging cols incl. b tails
                AT = atp.tile([P128, 8192], bf16, tag="AT")
                qpr = QH[:, a * P128:a * P128 + 256]
                po = ps_o.tile([P128, 512], f32, tag="po")
                av_a = av_b = 0

                goff = 0
                while goff < ecols:
                    gw = min(512, ecols - goff)
                    ps2 = ps_e.tile([P128, 512], f32, tag="ps2")
                    seg = goff
                    while seg < goff + gw:
                        kb = seg // 256
                        if kb < nfull:
                            nc.tensor.matmul(
                                ps2[:, seg - goff:seg - goff + 256],
                                KH[:, kb * P128:(kb + 1) * P128], qpr,
                                start=True, stop=True)
                        else:
                            # two b-only tail blocks, 128 cols each
                            for tix in range(2):
                                kb2 = nfull + tix
                                so = seg - goff + tix * P128
                                nc.tensor.matmul(
                                    ps2[:, so:so + P128],
                                    KH[:, kb2 * P128:(kb2 + 1) * P128],
                                    QH[:, b * P128:(b + 1) * P128],
                                    start=True, stop=True)
                        seg += 256
                        yield
                    # causal masks on PSUM before exp
                    lo, hi = goff, goff + gw
                    m0 = (4 * i) * 256          # a-half of block 4i
                    if lo <= m0 < hi:
                        nc.vector.tensor_add(
                            ps2[:, m0 - goff:m0 - goff + P128],
                            ps2[:, m0 - goff:m0 - goff + P128],
                            mskT[:, 0:P128])
                    m1 = (4 * i + 1) * 256      # a-half of block 4i+1
                    if lo <= m1 < hi:
                        nc.vector.tensor_add(
                            ps2[:, m1 - goff:m1 - goff + P128],
                            ps2[:, m1 - goff:m1 - goff + P128],
                            mskT[:, P128:256])
                    mt = nfull * 256            # b tails
                    if lo <= mt < hi:
                        nc.vector.tensor_add(
                            ps2[:, mt - goff:mt - goff + 256],
                            ps2[:, mt - goff:mt - goff + 256],
                            mskT)
                    nc.scalar.activation(AT[:, goff:goff + gw], ps2[:, :gw],
                                         ACTF.Exp, bias=0.0, scale=8.0)
                    yield
                    # AV for slot a over the blocks this group completed
                    # (b's chain must wait: one pending PSUM accumulation
                    # group per zero region)
                    while (av_a + 1) * 256 <= goff + gw and av_a < nfull:
                        blk = av_a
                        nc.tensor.matmul(
                            po[:, :DV], AT[:, blk * 256:blk * 256 + P128],
                            Vsb[:, blk * DV:(blk + 1) * DV],
                            start=(blk == 0), stop=(blk == nfull - 1))
                        av_a += 1
                    yield
                    goff += gw

                for blk in range(nfull + 2):
                    if blk < nfull:
                        st = AT[:, blk * 256 + P128:(blk + 1) * 256]
                    else:
                        st = AT[:, nfull * 256 + (blk - nfull) * P128:
                                nfull * 256 + (blk - nfull + 1) * P128]
                    nc.tensor.matmul(
                        po[:, DV:2 * DV], st,
                        Vsb[:, blk * DV:(blk + 1) * DV],
                        start=(blk == 0), stop=(blk == nfull + 1))
                    if blk % 4 == 3:
                        yield

                for s, col in ((a, 0), (b, DV)):
                    rl = small.tile([P128, 1], f32, tag="rl")
                    nc.vector.reciprocal(rl, po[:, col + D:col + D + 1])
                    yt = small.tile([P128, D], f32, tag="yt")
                    nc.vector.tensor_scalar_mul(yt, po[:, col:col + D], rl)
                    nc.sync.dma_start(out=y[s * P128:(s + 1) * P128, :],
                                      in_=yt)
                    yield

            def drain(gen):
                for _ in gen:
                    pass

            def interleave(g1, g2):
                """Pump g1 and g2 alternately (g2 twice per g1 step)."""
                done1 = done2 = False
                while not (done1 and done2):
                    if not done1:
                        done1 = next(g1, _SENTINEL) is _SENTINEL
                    if not done2:
                        for _ in range(3):
                            done2 = next(g2, _SENTINEL) is _SENTINEL
                            if done2:
                                break

            _SENTINEL = object()

            # ---- software-pipelined schedule ----
            load_kv(0)
            load_q(0)
            load_q(1)
            load_kv(1)
            prev = None
            for i in range(NPAIR):
                if i + 2 < NPAIR:
                    load_kv(i + 2)
                if i + 2 < NPAIR:
                    load_q(i + 2)
                proj_kv(i)
                proj_q(i)
                g1 = gen_pass1_pair(i)
                if prev is None:
                    drain(g1)
                else:
                    interleave(g1, prev)
                prev = gen_pass2_pair(i)
            drain(prev)

    nc.compile()
    return nc


def _get_nc():
    if "nc" not in _CACHED:
        _CACHED["nc"] = _build()
    return _CACHED["nc"]


def _prep_in_maps(x, Wq, bq, Wk, bk, Wv, bv):
    x = np.asarray(x, dtype=np.float32)
    Wq = np.asarray(Wq, dtype=np.float32)
    Wk = np.asarray(Wk, dtype=np.float32)
    Wv = np.asarray(Wv, dtype=np.float32)
    bq_ = np.asarray(bq, dtype=np.float32).reshape(D, 1)
    bk_ = np.asarray(bk, dtype=np.float32).reshape(D, 1)
    bv_ = np.asarray(bv, dtype=np.float32).reshape(D, 1)

    tri = np.triu(np.ones((P128, P128), np.float32), k=1) * NEG
    masks = []
    for P in range(2):
        mp = np.zeros((P128, 512), np.float32)
        if P == 0:
            mp[:, 256:384] = tri
            mp[:, 384:512] = NEG
        else:
            mp[:, 384:512] = tri
        masks.append(mp)

    masksT = []
    for P in range(2):
        mt = np.zeros((P128, 256), np.float32)
        mt[:, 0:128] = masks[P][:, 256:384].T
        mt[:, 128:256] = masks[P][:, 384:512].T
        masksT.append(mt)

    def hilo(a):
        """Stack [M, N] fp32 -> [2M, N] fp16 (hi rows, then residual rows)."""
        hi = a.astype(np.float16)
        lo = (a - hi.astype(np.float32)).astype(np.float16)
        return np.concatenate([hi, lo], axis=0)

    ones_row = np.ones((1, T), np.float32)
    wq2 = hilo(Wq)
    wkv2 = hilo(np.concatenate([Wk, Wv], axis=1))
    xTs = {}
    xTqs = {}
    in_maps = []
    for c in range(8):
        b, P = c // 2, c % 2
        if b not in xTs:
            xTs[b] = hilo(np.ascontiguousarray(x[b].T))
        if (b, P) not in xTqs:
            rows = ((np.arange(NSLOT) * 2 + P)[:, None] * P128
                    + np.arange(P128)[None, :]).reshape(-1)
            xTqs[(b, P)] = hilo(np.ascontiguousarray(x[b][rows].T))
        in_maps.append({
            "xT": xTs[b],
            "xTq": xTqs[(b, P)],
            "wqh": wq2, "wkv2": wkv2,
            "bq": bq_, "bk": bk_, "bv": bv_,
            "maskp": masks[P], "maskpT": masksT[P],
            "onesr": ones_row,
        })
    return in_maps


def _unshard(res):
    out = np.empty((B, T, D), np.float32)
    for c in range(8):
        b, P = c // 2, c % 2
        yl = res.results[c]["y"]
        for j in range(NSLOT):
            k = 2 * j + P
            out[b, k * P128:(k + 1) * P128] = yl[j * P128:(j + 1) * P128]
    return out


def kernel(x, Wq, bq, Wk, bk, Wv, bv):
    from concourse.bass_utils import run_bass_kernel_spmd

    in_maps = _prep_in_maps(x, Wq, bq, Wk, bk, Wv, bv)
    res = run_bass_kernel_spmd(_get_nc(), in_maps, core_ids=list(range(8)))
    _CACHED["last_results"] = res
    return _unshard(res)


if __name__ == "__main__":
    rng = np.random.default_rng(0)
    x = rng.standard_normal((B, T, C), dtype=np.float32)
    s = 1.0 / np.sqrt(C)
    Wq = rng.standard_normal((C, D), dtype=np.float32) * s
    Wk = rng.standard_normal((C, D), dtype=np.float32) * s
    Wv = rng.standard_normal((C, D), dtype=np.float32) * s
    z = np.zeros(D, np.float32)
    print(kernel(x, Wq, z, Wk, z, Wv, z).shape)


# revision 52
# speedup vs baseline: 2.1169x; 1.0776x over previous
"""Causal single-head attention (B=4, T=4096, C=512, D=64) on 8 TRN2 NeuronCores.

Sharding: core c -> (batch b = c // 2, parity P = c % 2).  Each batch's 32
q-tiles (128 rows each) are striped by parity: core (b, P) owns global q-tiles
k = 2j + P, j = 0..15.  Slot j's causal kv extent is padded to 256*(j+1) keys
(uniform across parities) and the last 256 key columns get a parity-specific
additive mask fed as input data, so one SPMD program serves all 8 cores.

Precision plan (calibrated on HW probes): projections run fp32 on PE (score
logits are x8-scaled, sigma ~64 -- tf32-rounded projections alone breach the
2e-2 gate).  Q-hat/K-hat are stored float32r (tf32, one rounding); both score
passes run f32r at 1 cycle/col (pass-2 slot-paired so the moving dim is 256).
A and V are bf16 (rounding after exp / on values is harmless).

Per-core dataflow, slots processed in pairs (a=2i, b=2i+1):
  stream per i: project K/V chunk i (keys 512i..512i+512), project Q pair i,
  then flash pair i (its kv extent 512(i+1) is now resident):
  1. pass-1 per slot: S = Q_s K^T in f32r, 512-wide PSUM chunks; row-max on
     DVE, pairs of chunks fused via tensor_tensor_reduce(max, max), causal
     mask folded into the last chunk via tensor_tensor_reduce(add, max);
     final negated max -> -m (fp16) -> [1,128] row via PE identity transpose
     -> QH partition 64 (f32r).
  2. pass-2 per pair: S^T - m via the 65-deep contraction [K^T;1]^T[Q^T;-m],
     moving = 256 q-cols (both slots) per key block; b-only tail blocks
     (last 512 keys) moving 128.  Transposed causal masks added on PSUM
     (DVE), one ACT exp (scale=8) per 1024-wide group writes A^T bf16.
  3. AV per slot: po[128, 65] += A^T_block^T V-hat_block (bf16); column 64
     accumulates l via V-hat's ones column; y = O / l on DVE.
"""

import numpy as np

B, T, C, D = 4, 4096, 512, 64
P128 = 128
NSLOT = 16          # q-tile slots per core
NPAIR = NSLOT // 2
TQ = NSLOT * P128   # 2048 q rows per core
NEG = -1.0e30

_CACHED = {}


def _build():
    import concourse.mybir as mybir
    from concourse import bacc
    from concourse.tile import TileContext
    from concourse.masks import make_identity

    f32 = mybir.dt.float32
    f32r = mybir.dt.float32r
    f16 = mybir.dt.float16
    bf16 = mybir.dt.bfloat16
    AX = mybir.AxisListType.X
    ALU = mybir.AluOpType
    ACTF = mybir.ActivationFunctionType

    nc = bacc.Bacc("TRN2", target_bir_lowering=False, debug=False,
                   enable_asserts=False, num_devices=8)

    xT = nc.dram_tensor("xT", [2 * C, T], f16, kind="ExternalInput").ap()
    xTq = nc.dram_tensor("xTq", [2 * C, TQ], f16, kind="ExternalInput").ap()
    wqh = nc.dram_tensor("wqh", [2 * C, D], f16, kind="ExternalInput").ap()
    wkv2 = nc.dram_tensor("wkv2", [2 * C, 2 * D], f16, kind="ExternalInput").ap()
    bq = nc.dram_tensor("bq", [D, 1], f32, kind="ExternalInput").ap()
    bk = nc.dram_tensor("bk", [D, 1], f32, kind="ExternalInput").ap()
    bv = nc.dram_tensor("bv", [D, 1], f32, kind="ExternalInput").ap()
    maskp = nc.dram_tensor("maskp", [P128, 512], f32, kind="ExternalInput").ap()
    maskpT = nc.dram_tensor("maskpT", [P128, 256], f32, kind="ExternalInput").ap()
    onesr = nc.dram_tensor("onesr", [1, T], f32r, kind="ExternalInput").ap()
    y = nc.dram_tensor("y", [TQ, D], f32, kind="ExternalOutput").ap()
    DV = D + 1  # V tiles carry a ones column

    with TileContext(nc) as tc:
        with (
            tc.tile_pool(name="singles", bufs=1) as singles,
            tc.tile_pool(name="xin", bufs=3) as xin,
            tc.tile_pool(name="qin", bufs=2) as qin,
            tc.tile_pool(name="atp", bufs=2) as atp,
            tc.tile_pool(name="vwork", bufs=2) as vwork,
            tc.tile_pool(name="small", bufs=4) as small,
            tc.tile_pool(name="ps_s", bufs=3, space="PSUM") as ps_s,
            tc.tile_pool(name="ps_e", bufs=2, space="PSUM") as ps_e,
            tc.tile_pool(name="ps_o", bufs=2, space="PSUM") as ps_o,
            tc.tile_pool(name="ps_m", bufs=1, space="PSUM") as ps_m,
        ):
            # ---- resident constants ----
            wqs = singles.tile([P128, 8, D], f16, tag="wqs")
            wkv = singles.tile([P128, 8, 2 * D], f16, tag="wkv")
            nc.sync.dma_start(out=wkv, in_=wkv2.rearrange("(c p) d -> p c d", p=P128))
            nc.sync.dma_start(out=wqs, in_=wqh.rearrange("(c p) d -> p c d", p=P128))
            bqs = singles.tile([D, 1], f32, tag="bqs")
            bks = singles.tile([D, 1], f32, tag="bks")
            bvs = singles.tile([D, 1], f32, tag="bvs")
            nc.sync.dma_start(out=bqs, in_=bq)
            nc.sync.dma_start(out=bks, in_=bk)
            nc.sync.dma_start(out=bvs, in_=bv)
            msk = singles.tile([P128, 512], f32, tag="msk")
            mskT = singles.tile([P128, 256], f32, tag="mskT")
            identb = singles.tile([P128, P128], bf16, tag="identb")
            make_identity(nc, identb)
            identh = singles.tile([P128, P128], f16, tag="identh")
            make_identity(nc, identh)

            # K-hat [K^T; ones] / Q-hat [Q^T; -m] in f32r (tf32-rounded once)
            KH = singles.tile([D + 1, T], f32r, tag="KH")
            QH = singles.tile([D + 1, TQ], f32r, tag="QH")

            Vsb = singles.tile([P128, (T // P128) * DV], bf16, tag="Vsb")
            nc.vector.memset(Vsb, 1.0)   # ones column survives V writes
            scr = singles.tile([1, 1], f32, tag="scr")
            nc.vector.memset(scr, 0.0)
            nc.scalar.activation(scr, scr, ACTF.Exp, bias=0.0, scale=1.0)
            # keep PE continuously busy through the DMA-bound startup so the
            # p-state ramp reaches full clock before real work arrives
            warm = ps_m.tile([P128, 512], f32, tag="pm")
            for _ in range(64):
                nc.tensor.matmul(warm[:, :P128], identb, identb,
                                 start=True, stop=True)

            xts, xqs = {}, {}

            def load_kv(t8):
                xts[t8] = xin.tile([P128, 8, 512], f16, tag="xt", name=f"xt{t8}")
                nc.sync.dma_start(
                    out=xts[t8],
                    in_=xT[:, t8 * 512:(t8 + 1) * 512].rearrange(
                        "(c p) n -> p c n", p=P128))

            def load_q(i):
                # pair i's q columns: [256*i, 256*i+256)
                xqs[i] = qin.tile([P128, 8, 256], f16, tag="xq", name=f"xq{i}")
                nc.sync.dma_start(
                    out=xqs[i],
                    in_=xTq[:, i * 256:(i + 1) * 256].rearrange(
                        "(c p) n -> p c n", p=P128))

            def proj_kv(t8):
                xt = xts.pop(t8)
                kvps = ps_s.tile([P128, 512], f32, tag="ps")
                pairs = ([(c, c) for c in range(4)]
                         + [(c, c + 4) for c in range(4)]
                         + [(c + 4, c) for c in range(4)])
                for n, (wc, xc) in enumerate(pairs):
                    nc.tensor.matmul(kvps, wkv[:, wc, :], xt[:, xc, :],
                                     start=(n == 0), stop=(n == len(pairs) - 1))
                nc.scalar.activation(KH[:D, t8 * 512:(t8 + 1) * 512],
                                     kvps[:D, :], ACTF.Identity,
                                     bias=bks, scale=1.0)
                vtmp = vwork.tile([D, 512], bf16, tag="vtmp")
                nc.scalar.activation(vtmp, kvps[D:, :], ACTF.Identity,
                                     bias=bvs, scale=1.0)
                # transpose V^T [64, 128] blocks -> bf16 V-hat [128, 65] tiles
                for i in range(4):
                    t = t8 * 4 + i
                    pt = ps_m.tile([P128, 512], bf16, tag="pm")
                    nc.tensor.transpose(pt[:, :D],
                                        vtmp[:, i * P128:(i + 1) * P128],
                                        identb[:D, :D])
                    nc.vector.tensor_copy(Vsb[:, t * DV:t * DV + D], pt[:, :D])

            def proj_q(i):
                # pair i's Q: 256 columns
                xt = xqs.pop(i)
                qps = ps_s.tile([P128, 512], f32, tag="ps")
                pairs = ([(c, c) for c in range(4)]
                         + [(c, c + 4) for c in range(4)]
                         + [(c + 4, c) for c in range(4)])
                for n, (wc, xc) in enumerate(pairs):
                    nc.tensor.matmul(qps[:D, :256], wqs[:, wc, :], xt[:, xc, :],
                                     start=(n == 0), stop=(n == len(pairs) - 1))
                nc.scalar.activation(QH[:D, i * 256:(i + 1) * 256],
                                     qps[:D, :256], ACTF.Identity,
                                     bias=bqs, scale=1.0)

            def gen_pass1_pair(i):
                """Row max per slot of pair i; writes -m into QH.  Yields
                between emission units so pass-2 of the previous pair can be
                interleaved into the in-order PE queue (keeps PE busy while
                DVE drains pass-1 PSUM chunks; preserves the p-state ramp)."""
                for s in (2 * i, 2 * i + 1):
                    ncols = 256 * (s + 1)
                    chunks = [(off, min(512, ncols - off))
                              for off in range(0, ncols, 512)]
                    mgp = small.tile([P128, 8], f32, tag="mgp")
                    ci = 0
                    qsl = QH[:D, s * P128:(s + 1) * P128]
                    for (off, sw) in chunks[:-1]:
                        ps = ps_s.tile([P128, 512], f32, tag="ps")
                        nc.tensor.matmul(ps, qsl, KH[:D, off:off + sw],
                                         start=True, stop=True)
                        nc.vector.reduce_max(mgp[:, ci:ci + 1], ps, axis=AX)
                        ci += 1
                        yield
                    off, sw = chunks[-1]
                    ps = ps_s.tile([P128, 512], f32, tag="ps")
                    nc.tensor.matmul(ps[:, :sw], qsl, KH[:D, off:off + sw],
                                     start=True, stop=True)
                    nc.vector.tensor_add(ps[:, :sw], ps[:, :sw],
                                         msk[:, 512 - sw:])
                    nc.vector.reduce_max(mgp[:, ci:ci + 1], ps[:, :sw],
                                         axis=AX)
                    ci += 1
                    yield
                    mrunh = small.tile([P128, 1], bf16, tag="mrunh")
                    nc.vector.reduce_max(mrunh, mgp[:, :ci], axis=AX,
                                         negate=True)
                    # -m -> [1,128] row via PE transpose -> QH partition 64
                    pm = ps_m.tile([P128, 512], bf16, tag="pm")
                    nc.tensor.transpose(pm[:1, :P128], mrunh, identb)
                    nc.vector.tensor_copy(
                        QH[D:D + 1, s * P128:(s + 1) * P128], pm[:1, :P128])
                    yield

            def gen_pass2_pair(i):
                a, b = 2 * i, 2 * i + 1
                nfull = 4 * i + 2          # key blocks where both slots attend
                ecols = nfull * 256 + 256  # pass-2 staging cols incl. b tails
                AT = atp.tile([P128, 8192], bf16, tag="AT")
                qpr = QH[:, a * P128:a * P128 + 256]
                po = ps_o.tile([P128, 512], f32, tag="po", name="po_a")
                pob = ps_o.tile([P128, 512], f32, tag="po", name="po_b")
                av_a = av_b = 0

                last_goff = (ecols - 1) // 512 * 512
                goff = 0
                while goff < ecols:
                    gw = min(512, ecols - goff)
                    ps2 = ps_e.tile([P128, 512], f32, tag="ps2")
                    seg = goff
                    while seg < goff + gw:
                        kb = seg // 256
                        if kb < nfull:
                            nc.tensor.matmul(
                                ps2[:, seg - goff:seg - goff + 256],
                                KH[:, kb * P128:(kb + 1) * P128], qpr,
                                start=True, stop=True)
                        else:
                            # two b-only tail blocks, 128 cols each
                            for tix in range(2):
                                kb2 = nfull + tix
                                so = seg - goff + tix * P128
                                nc.tensor.matmul(
                                    ps2[:, so:so + P128],
                                    KH[:, kb2 * P128:(kb2 + 1) * P128],
                                    QH[:, b * P128:(b + 1) * P128],
                                    start=True, stop=True)
                        seg += 256
                        yield
                    # causal masks on PSUM before exp
                    lo, hi = goff, goff + gw
                    m0 = (4 * i) * 256          # a-half of block 4i
                    if lo <= m0 < hi:
                        nc.vector.tensor_add(
                            ps2[:, m0 - goff:m0 - goff + P128],
                            ps2[:, m0 - goff:m0 - goff + P128],
                            mskT[:, 0:P128])
                    m1 = (4 * i + 1) * 256      # a-half of block 4i+1
                    if lo <= m1 < hi:
                        nc.vector.tensor_add(
                            ps2[:, m1 - goff:m1 - goff + P128],
                            ps2[:, m1 - goff:m1 - goff + P128],
                            mskT[:, P128:256])
                    mt = nfull * 256            # b tails
                    if lo <= mt < hi:
                        nc.vector.tensor_add(
                            ps2[:, mt - goff:mt - goff + 256],
                            ps2[:, mt - goff:mt - goff + 256],
                            mskT)
                    nc.scalar.activation(AT[:, goff:goff + gw], ps2[:, :gw],
                                         ACTF.Exp, bias=0.0, scale=8.0)
                    yield
                    # AV incrementally for both slots (separate PSUM banks);
                    # the FINAL group's AVs are deferred into the next pair's
                    # stream so the end-of-pair exp latency is covered
                    if goff < last_goff:
                        done = goff + gw
                        while (av_a + 1) * 256 <= done and av_a < nfull:
                            blk = av_a
                            nc.tensor.matmul(
                                po[:, :DV], AT[:, blk * 256:blk * 256 + P128],
                                Vsb[:, blk * DV:(blk + 1) * DV],
                                start=(blk == 0), stop=(blk == nfull - 1))
                            av_a += 1
                        while av_b < nfull and (av_b + 1) * 256 <= done:
                            blk = av_b
                            st = AT[:, blk * 256 + P128:(blk + 1) * 256]
                            nc.tensor.matmul(
                                pob[:, :DV], st,
                                Vsb[:, blk * DV:(blk + 1) * DV],
                                start=(blk == 0), stop=(blk == nfull + 1))
                            av_b += 1
                    yield
                    goff += gw

                def tail(a0=av_a, b0=av_b):
                    blk = a0
                    while blk < nfull:
                        nc.tensor.matmul(
                            po[:, :DV], AT[:, blk * 256:blk * 256 + P128],
                            Vsb[:, blk * DV:(blk + 1) * DV],
                            start=(blk == 0), stop=(blk == nfull - 1))
                        blk += 1
                    blk = b0
                    while blk < nfull + 2:
                        if blk < nfull:
                            st = AT[:, blk * 256 + P128:(blk + 1) * 256]
                        else:
                            st = AT[:, nfull * 256 + (blk - nfull) * P128:
                                    nfull * 256 + (blk - nfull + 1) * P128]
                        nc.tensor.matmul(
                            pob[:, :DV], st,
                            Vsb[:, blk * DV:(blk + 1) * DV],
                            start=(blk == 0), stop=(blk == nfull + 1))
                        blk += 1
                    for s, pot in ((a, po), (b, pob)):
                        rl = small.tile([P128, 1], f32, tag="rl")
                        nc.vector.reciprocal(rl, pot[:, D:D + 1])
                        yt = small.tile([P128, D], f32, tag="yt")
                        nc.vector.tensor_scalar_mul(yt, pot[:, :D], rl)
                        nc.sync.dma_start(out=y[s * P128:(s + 1) * P128, :],
                                          in_=yt)
                pend_finals.append(tail)
                yield

            def drain(gen):
                for _ in gen:
                    pass

            def interleave(g1, g2):
                """Pump g1 and g2 alternately (g2 twice per g1 step)."""
                done1 = done2 = False
                while not (done1 and done2):
                    if not done1:
                        done1 = next(g1, _SENTINEL) is _SENTINEL
                    if not done2:
                        for _ in range(3):
                            done2 = next(g2, _SENTINEL) is _SENTINEL
                            if done2:
                                break

            _SENTINEL = object()

            # ---- software-pipelined schedule ----
            load_kv(0)
            load_q(0)
            nc.sync.dma_start(out=msk, in_=maskp)
            nc.sync.dma_start(out=mskT, in_=maskpT)
            nc.sync.dma_start(out=KH[D:D + 1, :], in_=onesr)
            load_q(1)
            load_kv(1)
            prev = None
            pend_finals = []
            for i in range(NPAIR):
                if i + 2 < NPAIR:
                    load_kv(i + 2)
                if i + 2 < NPAIR:
                    load_q(i + 2)
                proj_kv(i)
                proj_q(i)
                g1 = gen_pass1_pair(i)
                if prev is None:
                    drain(g1)
                else:
                    interleave(g1, prev)
                while pend_finals:
                    pend_finals.pop(0)()
                prev = gen_pass2_pair(i)
            drain(prev)
            while pend_finals:
                pend_finals.pop(0)()

    nc.compile()
    return nc


def _get_nc():
    if "nc" not in _CACHED:
        _CACHED["nc"] = _build()
    return _CACHED["nc"]


def _prep_in_maps(x, Wq, bq, Wk, bk, Wv, bv):
    x = np.asarray(x, dtype=np.float32)
    Wq = np.asarray(Wq, dtype=np.float32)
    Wk = np.asarray(Wk, dtype=np.float32)
    Wv = np.asarray(Wv, dtype=np.float32)
    bq_ = np.asarray(bq, dtype=np.float32).reshape(D, 1)
    bk_ = np.asarray(bk, dtype=np.float32).reshape(D, 1)
    bv_ = np.asarray(bv, dtype=np.float32).reshape(D, 1)

    tri = np.triu(np.ones((P128, P128), np.float32), k=1) * NEG
    masks = []
    for P in range(2):
        mp = np.zeros((P128, 512), np.float32)
        if P == 0:
            mp[:, 256:384] = tri
            mp[:, 384:512] = NEG
        else:
            mp[:, 384:512] = tri
        masks.append(mp)

    masksT = []
    for P in range(2):
        mt = np.zeros((P128, 256), np.float32)
        mt[:, 0:128] = masks[P][:, 256:384].T
        mt[:, 128:256] = masks[P][:, 384:512].T
        masksT.append(mt)

    def hilo(a):
        """Stack [M, N] fp32 -> [2M, N] fp16 (hi rows, then residual rows)."""
        hi = a.astype(np.float16)
        lo = (a - hi.astype(np.float32)).astype(np.float16)
        return np.concatenate([hi, lo], axis=0)

    ones_row = np.ones((1, T), np.float32)
    wq2 = hilo(Wq)
    wkv2 = hilo(np.concatenate([Wk, Wv], axis=1))
    xTs = {}
    xTqs = {}
    in_maps = []
    for c in range(8):
        b, P = c // 2, c % 2
        if b not in xTs:
            xTs[b] = hilo(np.ascontiguousarray(x[b].T))
        if (b, P) not in xTqs:
            rows = ((np.arange(NSLOT) * 2 + P)[:, None] * P128
                    + np.arange(P128)[None, :]).reshape(-1)
            xTqs[(b, P)] = hilo(np.ascontiguousarray(x[b][rows].T))
        in_maps.append({
            "xT": xTs[b],
            "xTq": xTqs[(b, P)],
            "wqh": wq2, "wkv2": wkv2,
            "bq": bq_, "bk": bk_, "bv": bv_,
            "maskp": masks[P], "maskpT": masksT[P],
            "onesr": ones_row,
        })
    return in_maps


def _unshard(res):
    out = np.empty((B, T, D), np.float32)
    for c in range(8):
        b, P = c // 2, c % 2
        yl = res.results[c]["y"]
        for j in range(NSLOT):
            k = 2 * j + P
            out[b, k * P128:(k + 1) * P128] = yl[j * P128:(j + 1) * P128]
    return out


def kernel(x, Wq, bq, Wk, bk, Wv, bv):
    from concourse.bass_utils import run_bass_kernel_spmd

    in_maps = _prep_in_maps(x, Wq, bq, Wk, bk, Wv, bv)
    res = run_bass_kernel_spmd(_get_nc(), in_maps, core_ids=list(range(8)))
    _CACHED["last_results"] = res
    return _unshard(res)


if __name__ == "__main__":
    rng = np.random.default_rng(0)
    x = rng.standard_normal((B, T, C), dtype=np.float32)
    s = 1.0 / np.sqrt(C)
    Wq = rng.standard_normal((C, D), dtype=np.float32) * s
    Wk = rng.standard_normal((C, D), dtype=np.float32) * s
    Wv = rng.standard_normal((C, D), dtype=np.float32) * s
    z = np.zeros(D, np.float32)
    print(kernel(x, Wq, z, Wk, z, Wv, z).shape)


# revision 59
# speedup vs baseline: 2.1286x; 1.0055x over previous
"""Causal single-head attention (B=4, T=4096, C=512, D=64) on 8 TRN2 NeuronCores.

Sharding: core c -> (batch b = c // 2, parity P = c % 2).  Each batch's 32
q-tiles (128 rows each) are striped by parity: core (b, P) owns global q-tiles
k = 2j + P, j = 0..15.  Slot j's causal kv extent is padded to 256*(j+1) keys
(uniform across parities) and the last 256 key columns get a parity-specific
additive mask fed as input data, so one SPMD program serves all 8 cores.

Precision plan (calibrated on HW probes): projections run fp32 on PE (score
logits are x8-scaled, sigma ~64 -- tf32-rounded projections alone breach the
2e-2 gate).  Q-hat/K-hat are stored float32r (tf32, one rounding); both score
passes run f32r at 1 cycle/col (pass-2 slot-paired so the moving dim is 256).
A and V are bf16 (rounding after exp / on values is harmless).

Per-core dataflow, slots processed in pairs (a=2i, b=2i+1):
  stream per i: project K/V chunk i (keys 512i..512i+512), project Q pair i,
  then flash pair i (its kv extent 512(i+1) is now resident):
  1. pass-1 per slot: S = Q_s K^T in f32r, 512-wide PSUM chunks; row-max on
     DVE, pairs of chunks fused via tensor_tensor_reduce(max, max), causal
     mask folded into the last chunk via tensor_tensor_reduce(add, max);
     final negated max -> -m (fp16) -> [1,128] row via PE identity transpose
     -> QH partition 64 (f32r).
  2. pass-2 per pair: S^T - m via the 65-deep contraction [K^T;1]^T[Q^T;-m],
     moving = 256 q-cols (both slots) per key block; b-only tail blocks
     (last 512 keys) moving 128.  Transposed causal masks added on PSUM
     (DVE), one ACT exp (scale=8) per 1024-wide group writes A^T bf16.
  3. AV per slot: po[128, 65] += A^T_block^T V-hat_block (bf16); column 64
     accumulates l via V-hat's ones column; y = O / l on DVE.
"""

import numpy as np

B, T, C, D = 4, 4096, 512, 64
P128 = 128
NSLOT = 16          # q-tile slots per core
NPAIR = NSLOT // 2
TQ = NSLOT * P128   # 2048 q rows per core
NEG = -1.0e30

_CACHED = {}


def _build():
    import concourse.mybir as mybir
    from concourse import bacc
    from concourse.tile import TileContext
    from concourse.masks import make_identity

    f32 = mybir.dt.float32
    f32r = mybir.dt.float32r
    f16 = mybir.dt.float16
    bf16 = mybir.dt.bfloat16
    AX = mybir.AxisListType.X
    ALU = mybir.AluOpType
    ACTF = mybir.ActivationFunctionType

    nc = bacc.Bacc("TRN2", target_bir_lowering=False, debug=False,
                   enable_asserts=False, num_devices=8)

    xT = nc.dram_tensor("xT", [2 * C, T], f16, kind="ExternalInput").ap()
    xTq = nc.dram_tensor("xTq", [2 * C, TQ], f16, kind="ExternalInput").ap()
    wqh = nc.dram_tensor("wqh", [2 * C, D], f16, kind="ExternalInput").ap()
    wkv2 = nc.dram_tensor("wkv2", [2 * C, 2 * D], f16, kind="ExternalInput").ap()
    bq = nc.dram_tensor("bq", [D, 1], f32, kind="ExternalInput").ap()
    bk = nc.dram_tensor("bk", [D, 1], f32, kind="ExternalInput").ap()
    bv = nc.dram_tensor("bv", [D, 1], f32, kind="ExternalInput").ap()
    maskp = nc.dram_tensor("maskp", [P128, 512], f32, kind="ExternalInput").ap()
    maskpT = nc.dram_tensor("maskpT", [P128, 256], f32, kind="ExternalInput").ap()
    onesr = nc.dram_tensor("onesr", [1, T], f32r, kind="ExternalInput").ap()
    y = nc.dram_tensor("y", [TQ, D], f32, kind="ExternalOutput").ap()
    DV = D + 1  # V tiles carry a ones column

    with TileContext(nc) as tc:
        with (
            tc.tile_pool(name="singles", bufs=1) as singles,
            tc.tile_pool(name="xin", bufs=3) as xin,
            tc.tile_pool(name="qin", bufs=2) as qin,
            tc.tile_pool(name="atp", bufs=2) as atp,
            tc.tile_pool(name="vwork", bufs=3) as vwork,
            tc.tile_pool(name="small", bufs=6) as small,
            tc.tile_pool(name="ps_s", bufs=3, space="PSUM") as ps_s,
            tc.tile_pool(name="ps_e", bufs=2, space="PSUM") as ps_e,
            tc.tile_pool(name="ps_o", bufs=2, space="PSUM") as ps_o,
            tc.tile_pool(name="ps_m", bufs=1, space="PSUM") as ps_m,
        ):
            # ---- resident constants ----
            wqs = singles.tile([P128, 8, D], f16, tag="wqs")
            wkv = singles.tile([P128, 8, 2 * D], f16, tag="wkv")
            nc.sync.dma_start(out=wkv, in_=wkv2.rearrange("(c p) d -> p c d", p=P128))
            nc.sync.dma_start(out=wqs, in_=wqh.rearrange("(c p) d -> p c d", p=P128))
            bqs = singles.tile([D, 1], f32, tag="bqs")
            bks = singles.tile([D, 1], f32, tag="bks")
            bvs = singles.tile([D, 1], f32, tag="bvs")
            nc.sync.dma_start(out=bqs, in_=bq)
            nc.sync.dma_start(out=bks, in_=bk)
            nc.sync.dma_start(out=bvs, in_=bv)
            msk = singles.tile([P128, 512], f32, tag="msk")
            mskT = singles.tile([P128, 256], f32, tag="mskT")
            identb = singles.tile([P128, P128], bf16, tag="identb")
            make_identity(nc, identb)
            identh = singles.tile([P128, P128], f16, tag="identh")
            make_identity(nc, identh)

            # K-hat [K^T; ones] / Q-hat [Q^T; -m] in f32r (tf32-rounded once)
            KH = singles.tile([D + 1, T], f32r, tag="KH")
            QH = singles.tile([D + 1, TQ], f32r, tag="QH")

            Vsb = singles.tile([P128, (T // P128) * DV], bf16, tag="Vsb")
            nc.gpsimd.memset(Vsb, 1.0)   # ones column survives V writes
            scr = singles.tile([1, 1], f32, tag="scr")
            nc.gpsimd.memset(scr, 0.0)
            nc.scalar.activation(scr, scr, ACTF.Exp, bias=0.0, scale=1.0)
            # keep PE continuously busy through the DMA-bound startup so the
            # p-state ramp reaches full clock before real work arrives
            warm = ps_m.tile([P128, 512], f32, tag="pm")
            for _ in range(64):
                nc.tensor.matmul(warm[:, :P128], identb, identb,
                                 start=True, stop=True)

            xts, xqs = {}, {}

            def load_kv(t8):
                xts[t8] = xin.tile([P128, 8, 512], f16, tag="xt", name=f"xt{t8}")
                nc.sync.dma_start(
                    out=xts[t8],
                    in_=xT[:, t8 * 512:(t8 + 1) * 512].rearrange(
                        "(c p) n -> p c n", p=P128))

            def load_q(i):
                # pair i's q columns: [256*i, 256*i+256)
                xqs[i] = qin.tile([P128, 8, 256], f16, tag="xq", name=f"xq{i}")
                nc.sync.dma_start(
                    out=xqs[i],
                    in_=xTq[:, i * 256:(i + 1) * 256].rearrange(
                        "(c p) n -> p c n", p=P128))

            def proj_kv(t8):
                xt = xts.pop(t8)
                kvps = ps_s.tile([P128, 512], f32, tag="ps")
                pairs = ([(c, c) for c in range(4)]
                         + [(c, c + 4) for c in range(4)]
                         + [(c + 4, c) for c in range(4)])
                for n, (wc, xc) in enumerate(pairs):
                    nc.tensor.matmul(kvps, wkv[:, wc, :], xt[:, xc, :],
                                     start=(n == 0), stop=(n == len(pairs) - 1))
                nc.scalar.activation(KH[:D, t8 * 512:(t8 + 1) * 512],
                                     kvps[:D, :], ACTF.Identity,
                                     bias=bks, scale=1.0)
                vtmp = vwork.tile([D, 512], bf16, tag="vtmp")
                nc.scalar.activation(vtmp, kvps[D:, :], ACTF.Identity,
                                     bias=bvs, scale=1.0)
                # transpose V^T [64, 128] blocks -> bf16 V-hat [128, 65] tiles
                for i in range(4):
                    t = t8 * 4 + i
                    pt = ps_m.tile([P128, 512], bf16, tag="pm")
                    nc.tensor.transpose(pt[:, :D],
                                        vtmp[:, i * P128:(i + 1) * P128],
                                        identb[:D, :D])
                    nc.vector.tensor_copy(Vsb[:, t * DV:t * DV + D], pt[:, :D])

            def proj_q(i):
                # pair i's Q: 256 columns
                xt = xqs.pop(i)
                qps = ps_s.tile([P128, 512], f32, tag="ps")
                pairs = ([(c, c) for c in range(4)]
                         + [(c, c + 4) for c in range(4)]
                         + [(c + 4, c) for c in range(4)])
                for n, (wc, xc) in enumerate(pairs):
                    nc.tensor.matmul(qps[:D, :256], wqs[:, wc, :], xt[:, xc, :],
                                     start=(n == 0), stop=(n == len(pairs) - 1))
                nc.scalar.activation(QH[:D, i * 256:(i + 1) * 256],
                                     qps[:D, :256], ACTF.Identity,
                                     bias=bqs, scale=1.0)

            def gen_pass1_pair(i):
                """Row max per slot of pair i; writes -m into QH.  Yields
                between emission units so pass-2 of the previous pair can be
                interleaved into the in-order PE queue (keeps PE busy while
                DVE drains pass-1 PSUM chunks; preserves the p-state ramp)."""
                for s in (2 * i, 2 * i + 1):
                    ncols = 256 * (s + 1)
                    chunks = [(off, min(512, ncols - off))
                              for off in range(0, ncols, 512)]
                    mgp = small.tile([P128, 8], f32, tag="mgp")
                    ci = 0
                    qsl = QH[:D, s * P128:(s + 1) * P128]
                    for (off, sw) in chunks[:-1]:
                        ps = ps_s.tile([P128, 512], f32, tag="ps")
                        nc.tensor.matmul(ps, qsl, KH[:D, off:off + sw],
                                         start=True, stop=True)
                        nc.vector.reduce_max(mgp[:, ci:ci + 1], ps, axis=AX)
                        ci += 1
                        yield
                    off, sw = chunks[-1]
                    ps = ps_s.tile([P128, 512], f32, tag="ps")
                    nc.tensor.matmul(ps[:, :sw], qsl, KH[:D, off:off + sw],
                                     start=True, stop=True)
                    nc.vector.tensor_add(ps[:, :sw], ps[:, :sw],
                                         msk[:, 512 - sw:])
                    nc.vector.reduce_max(mgp[:, ci:ci + 1], ps[:, :sw],
                                         axis=AX)
                    ci += 1
                    yield
                    mrunh = small.tile([P128, 1], bf16, tag="mrunh")
                    nc.vector.reduce_max(mrunh, mgp[:, :ci], axis=AX,
                                         negate=True)
                    # -m -> [1,128] row via PE transpose -> QH partition 64
                    pm = ps_m.tile([P128, 512], bf16, tag="pm")
                    nc.tensor.transpose(pm[:1, :P128], mrunh, identb)
                    nc.vector.tensor_copy(
                        QH[D:D + 1, s * P128:(s + 1) * P128], pm[:1, :P128])
                    yield

            def gen_pass2_pair(i):
                a, b = 2 * i, 2 * i + 1
                nfull = 4 * i + 2          # key blocks where both slots attend
                ecols = nfull * 256 + 256  # pass-2 staging cols incl. b tails
                while pend_finals:
                    pend_finals.pop(0)()
                AT = atp.tile([P128, 8192], bf16, tag="AT")
                qpr = QH[:, a * P128:a * P128 + 256]
                po = ps_o.tile([P128, 512], f32, tag="po", name="po_a")
                pob = ps_o.tile([P128, 512], f32, tag="po", name="po_b")
                av_a = av_b = 0

                last_goff = max(0, (ecols - 1) // 512 * 512 - 512)
                goff = 0
                while goff < ecols:
                    gw = min(512, ecols - goff)
                    ps2 = ps_e.tile([P128, 512], f32, tag="ps2")
                    seg = goff
                    while seg < goff + gw:
                        kb = seg // 256
                        if kb < nfull:
                            nc.tensor.matmul(
                                ps2[:, seg - goff:seg - goff + 256],
                                KH[:, kb * P128:(kb + 1) * P128], qpr,
                                start=True, stop=True)
                        else:
                            # two b-only tail blocks, 128 cols each
                            for tix in range(2):
                                kb2 = nfull + tix
                                so = seg - goff + tix * P128
                                nc.tensor.matmul(
                                    ps2[:, so:so + P128],
                                    KH[:, kb2 * P128:(kb2 + 1) * P128],
                                    QH[:, b * P128:(b + 1) * P128],
                                    start=True, stop=True)
                        seg += 256
                        yield
                    # causal masks on PSUM before exp
                    lo, hi = goff, goff + gw
                    m0 = (4 * i) * 256          # a-half of block 4i
                    if lo <= m0 < hi:
                        nc.vector.tensor_add(
                            ps2[:, m0 - goff:m0 - goff + P128],
                            ps2[:, m0 - goff:m0 - goff + P128],
                            mskT[:, 0:P128])
                    m1 = (4 * i + 1) * 256      # a-half of block 4i+1
                    if lo <= m1 < hi:
                        nc.vector.tensor_add(
                            ps2[:, m1 - goff:m1 - goff + P128],
                            ps2[:, m1 - goff:m1 - goff + P128],
                            mskT[:, P128:256])
                    mt = nfull * 256            # b tails
                    if lo <= mt < hi:
                        nc.vector.tensor_add(
                            ps2[:, mt - goff:mt - goff + 256],
                            ps2[:, mt - goff:mt - goff + 256],
                            mskT)
                    nc.scalar.activation(AT[:, goff:goff + gw], ps2[:, :gw],
                                         ACTF.Exp, bias=0.0, scale=8.0)
                    yield
                    # AV incrementally for both slots (separate PSUM banks);
                    # the FINAL group's AVs are deferred into the next pair's
                    # stream so the end-of-pair exp latency is covered
                    if goff < last_goff:
                        done = goff + gw
                        while (av_a + 1) * 256 <= done and av_a < nfull:
                            blk = av_a
                            nc.tensor.matmul(
                                po[:, :DV], AT[:, blk * 256:blk * 256 + P128],
                                Vsb[:, blk * DV:(blk + 1) * DV],
                                start=(blk == 0), stop=(blk == nfull - 1))
                            av_a += 1
                        while av_b < nfull and (av_b + 1) * 256 <= done:
                            blk = av_b
                            st = AT[:, blk * 256 + P128:(blk + 1) * 256]
                            nc.tensor.matmul(
                                pob[:, :DV], st,
                                Vsb[:, blk * DV:(blk + 1) * DV],
                                start=(blk == 0), stop=(blk == nfull + 1))
                            av_b += 1
                    yield
                    goff += gw

                def tail(a0=av_a, b0=av_b):
                    blk = a0
                    while blk < nfull:
                        nc.tensor.matmul(
                            po[:, :DV], AT[:, blk * 256:blk * 256 + P128],
                            Vsb[:, blk * DV:(blk + 1) * DV],
                            start=(blk == 0), stop=(blk == nfull - 1))
                        blk += 1
                    blk = b0
                    while blk < nfull + 2:
                        if blk < nfull:
                            st = AT[:, blk * 256 + P128:(blk + 1) * 256]
                        else:
                            st = AT[:, nfull * 256 + (blk - nfull) * P128:
                                    nfull * 256 + (blk - nfull + 1) * P128]
                        nc.tensor.matmul(
                            pob[:, :DV], st,
                            Vsb[:, blk * DV:(blk + 1) * DV],
                            start=(blk == 0), stop=(blk == nfull + 1))
                        blk += 1
                    for s, pot in ((a, po), (b, pob)):
                        rl = small.tile([P128, 1], f32, tag="rl")
                        nc.vector.reciprocal(rl, pot[:, D:D + 1])
                        yt = small.tile([P128, D], f32, tag="yt")
                        nc.vector.tensor_scalar_mul(yt, pot[:, :D], rl)
                        nc.sync.dma_start(out=y[s * P128:(s + 1) * P128, :],
                                          in_=yt)
                pend_finals.append(tail)
                yield

            def drain(gen):
                for _ in gen:
                    pass

            def interleave(g1, g2):
                """Pump g1 and g2 alternately (g2 twice per g1 step)."""
                done1 = done2 = False
                while not (done1 and done2):
                    if not done1:
                        done1 = next(g1, _SENTINEL) is _SENTINEL
                    if not done2:
                        for _ in range(3):
                            done2 = next(g2, _SENTINEL) is _SENTINEL
                            if done2:
                                break

            _SENTINEL = object()

            # ---- software-pipelined schedule ----
            load_kv(0)
            load_q(0)
            nc.sync.dma_start(out=msk, in_=maskp)
            load_kv(1)
            load_q(1)
            nc.sync.dma_start(out=mskT, in_=maskpT)
            nc.sync.dma_start(out=KH[D:D + 1, :], in_=onesr)
            prev = None
            pend_finals = []
            for i in range(NPAIR):
                if i + 2 < NPAIR:
                    load_kv(i + 2)
                if i + 2 < NPAIR:
                    load_q(i + 2)
                proj_kv(i)
                proj_q(i)
                g1 = gen_pass1_pair(i)
                if prev is None:
                    drain(g1)
                else:
                    interleave(g1, prev)
                prev = gen_pass2_pair(i)
            drain(prev)
            while pend_finals:
                pend_finals.pop(0)()

    nc.compile()
    return nc


def _get_nc():
    if "nc" not in _CACHED:
        _CACHED["nc"] = _build()
    return _CACHED["nc"]


def _prep_in_maps(x, Wq, bq, Wk, bk, Wv, bv):
    x = np.asarray(x, dtype=np.float32)
    Wq = np.asarray(Wq, dtype=np.float32)
    Wk = np.asarray(Wk, dtype=np.float32)
    Wv = np.asarray(Wv, dtype=np.float32)
    bq_ = np.asarray(bq, dtype=np.float32).reshape(D, 1)
    bk_ = np.asarray(bk, dtype=np.float32).reshape(D, 1)
    bv_ = np.asarray(bv, dtype=np.float32).reshape(D, 1)

    tri = np.triu(np.ones((P128, P128), np.float32), k=1) * NEG
    masks = []
    for P in range(2):
        mp = np.zeros((P128, 512), np.float32)
        if P == 0:
            mp[:, 256:384] = tri
            mp[:, 384:512] = NEG
        else:
            mp[:, 384:512] = tri
        masks.append(mp)

    masksT = []
    for P in range(2):
        mt = np.zeros((P128, 256), np.float32)
        mt[:, 0:128] = masks[P][:, 256:384].T
        mt[:, 128:256] = masks[P][:, 384:512].T
        masksT.append(mt)

    def hilo(a):
        """Stack [M, N] fp32 -> [2M, N] fp16 (hi rows, then residual rows)."""
        hi = a.astype(np.float16)
        lo = (a - hi.astype(np.float32)).astype(np.float16)
        return np.concatenate([hi, lo], axis=0)

    ones_row = np.ones((1, T), np.float32)
    wq2 = hilo(Wq)
    wkv2 = hilo(np.concatenate([Wk, Wv], axis=1))
    xTs = {}
    xTqs = {}
    in_maps = []
    for c in range(8):
        b, P = c // 2, c % 2
        if b not in xTs:
            xTs[b] = hilo(np.ascontiguousarray(x[b].T))
        if (b, P) not in xTqs:
            rows = ((np.arange(NSLOT) * 2 + P)[:, None] * P128
                    + np.arange(P128)[None, :]).reshape(-1)
            xTqs[(b, P)] = hilo(np.ascontiguousarray(x[b][rows].T))
        in_maps.append({
            "xT": xTs[b],
            "xTq": xTqs[(b, P)],
            "wqh": wq2, "wkv2": wkv2,
            "bq": bq_, "bk": bk_, "bv": bv_,
            "maskp": masks[P], "maskpT": masksT[P],
            "onesr": ones_row,
        })
    return in_maps


def _unshard(res):
    out = np.empty((B, T, D), np.float32)
    for c in range(8):
        b, P = c // 2, c % 2
        yl = res.results[c]["y"]
        for j in range(NSLOT):
            k = 2 * j + P
            out[b, k * P128:(k + 1) * P128] = yl[j * P128:(j + 1) * P128]
    return out


def kernel(x, Wq, bq, Wk, bk, Wv, bv):
    from concourse.bass_utils import run_bass_kernel_spmd

    in_maps = _prep_in_maps(x, Wq, bq, Wk, bk, Wv, bv)
    res = run_bass_kernel_spmd(_get_nc(), in_maps, core_ids=list(range(8)))
    _CACHED["last_results"] = res
    return _unshard(res)


if __name__ == "__main__":
    rng = np.random.default_rng(0)
    x = rng.standard_normal((B, T, C), dtype=np.float32)
    s = 1.0 / np.sqrt(C)
    Wq = rng.standard_normal((C, D), dtype=np.float32) * s
    Wk = rng.standard_normal((C, D), dtype=np.float32) * s
    Wv = rng.standard_normal((C, D), dtype=np.float32) * s
    z = np.zeros(D, np.float32)
    print(kernel(x, Wq, z, Wk, z, Wv, z).shape)


# revision 64
# speedup vs baseline: 2.1724x; 1.0206x over previous
"""Causal single-head attention (B=4, T=4096, C=512, D=64) on 8 TRN2 NeuronCores.

Sharding: core c -> (batch b = c // 2, parity P = c % 2).  Each batch's 32
q-tiles (128 rows each) are striped by parity: core (b, P) owns global q-tiles
k = 2j + P, j = 0..15.  Slot j's causal kv extent is padded to 256*(j+1) keys
(uniform across parities) and the last 256 key columns get a parity-specific
additive mask fed as input data, so one SPMD program serves all 8 cores.

Precision plan (calibrated on HW probes): the x8-scaled score logits
(sigma ~64) make softmax weights exquisitely sensitive, so projections run
at fp32 quality via a 3-pass fp16 hi/lo split (x and W shipped as stacked
f16 hi|lo pairs; x_hi*W_hi + x_lo*W_hi + x_hi*W_lo at 1 cyc/col each vs
fp32's 4).  Q-hat/K-hat are stored float32r (tf32, one rounding); both score
passes run f32r at 1 cycle/col (pass-2 slot-paired so the moving dim is 256).
A and V are bf16 (rounding after exp / on values is harmless).

Per-core dataflow, slots processed in pairs (a=2i, b=2i+1):
  stream per i: project K/V chunk i (keys 512i..512i+512), project Q pair i,
  then flash pair i (its kv extent 512(i+1) is now resident):
  1. pass-1 per slot: S = Q_s K^T in f32r, 512-wide PSUM chunks; row-max per
     chunk on DVE (causal mask added to the last chunk first); the negated
     final max -> -m (bf16) -> [1,128] row via PE identity transpose -> QH
     partition 64 (f32r).  Emission is software-pipelined: pair i's pass-1
     interleaves with pair i-1's pass-2 so the in-order PE queue never
     stalls on the PE->DVE->PE max chain (keeps the p-state ramp at full
     clock; dummy warm-up matmuls cover the DMA-bound startup).
  2. pass-2 per pair: S^T - m via the 65-deep contraction [K^T;1]^T[Q^T;-m],
     moving = 256 q-cols (both slots) per key block; b-only tail blocks
     (last 512 keys) moving 128.  Transposed causal masks added on PSUM
     (DVE), one ACT exp (scale=8) per 1024-wide group writes A^T bf16.
  3. AV per slot: po[128, 65] += A^T_block^T V-hat_block (bf16); column 64
     accumulates l via V-hat's ones column; y = O / l on DVE.
"""

import numpy as np

B, T, C, D = 4, 4096, 512, 64
P128 = 128
NSLOT = 16          # q-tile slots per core
NPAIR = NSLOT // 2
TQ = NSLOT * P128   # 2048 q rows per core
NEG = -1.0e30

_CACHED = {}


def _build():
    import concourse.mybir as mybir
    from concourse import bacc
    from concourse.tile import TileContext
    from concourse.masks import make_identity

    f32 = mybir.dt.float32
    f32r = mybir.dt.float32r
    f16 = mybir.dt.float16
    bf16 = mybir.dt.bfloat16
    AX = mybir.AxisListType.X
    ALU = mybir.AluOpType
    ACTF = mybir.ActivationFunctionType

    nc = bacc.Bacc("TRN2", target_bir_lowering=False, debug=False,
                   enable_asserts=False, num_devices=8)

    xT = nc.dram_tensor("xT", [2 * C, T], f16, kind="ExternalInput").ap()
    xTq = nc.dram_tensor("xTq", [2 * C, TQ], f16, kind="ExternalInput").ap()
    wqh = nc.dram_tensor("wqh", [2 * C, D], f16, kind="ExternalInput").ap()
    wkv2 = nc.dram_tensor("wkv2", [2 * C, 2 * D], f16, kind="ExternalInput").ap()
    bq = nc.dram_tensor("bq", [D, 1], f32, kind="ExternalInput").ap()
    bk = nc.dram_tensor("bk", [D, 1], f32, kind="ExternalInput").ap()
    bv = nc.dram_tensor("bv", [D, 1], f32, kind="ExternalInput").ap()
    maskp = nc.dram_tensor("maskp", [P128, 512], f32, kind="ExternalInput").ap()
    maskpT = nc.dram_tensor("maskpT", [P128, 256], f32, kind="ExternalInput").ap()
    onesr = nc.dram_tensor("onesr", [1, T], f32r, kind="ExternalInput").ap()
    y = nc.dram_tensor("y", [TQ, D], f32, kind="ExternalOutput").ap()
    DV = D + 1  # V tiles carry a ones column

    with TileContext(nc) as tc:
        with (
            tc.tile_pool(name="singles", bufs=1) as singles,
            tc.tile_pool(name="xin", bufs=3) as xin,
            tc.tile_pool(name="qin", bufs=2) as qin,
            tc.tile_pool(name="atp", bufs=2) as atp,
            tc.tile_pool(name="vwork", bufs=3) as vwork,
            tc.tile_pool(name="small", bufs=6) as small,
            tc.tile_pool(name="ps_s", bufs=3, space="PSUM") as ps_s,
            tc.tile_pool(name="ps_e", bufs=2, space="PSUM") as ps_e,
            tc.tile_pool(name="ps_o", bufs=2, space="PSUM") as ps_o,
            tc.tile_pool(name="ps_m", bufs=1, space="PSUM") as ps_m,
        ):
            # ---- resident constants ----
            wqs = singles.tile([P128, 8, D], f16, tag="wqs")
            wkv = singles.tile([P128, 8, 2 * D], f16, tag="wkv")
            nc.sync.dma_start(out=wkv, in_=wkv2.rearrange("(c p) d -> p c d", p=P128))
            nc.sync.dma_start(out=wqs, in_=wqh.rearrange("(c p) d -> p c d", p=P128))
            bqs = singles.tile([D, 1], f32, tag="bqs")
            bks = singles.tile([D, 1], f32, tag="bks")
            bvs = singles.tile([D, 1], f32, tag="bvs")
            nc.sync.dma_start(out=bqs, in_=bq)
            nc.sync.dma_start(out=bks, in_=bk)
            nc.sync.dma_start(out=bvs, in_=bv)
            msk = singles.tile([P128, 512], f32, tag="msk")
            mskT = singles.tile([P128, 256], f32, tag="mskT")
            identb = singles.tile([P128, P128], bf16, tag="identb")
            make_identity(nc, identb)
            identh = singles.tile([P128, P128], f16, tag="identh")
            make_identity(nc, identh)

            # K-hat [K^T; ones] / Q-hat [Q^T; -m] in f32r (tf32-rounded once)
            KH = singles.tile([D + 1, T], f32r, tag="KH")
            QH = singles.tile([D + 1, TQ], f32r, tag="QH")

            Vsb = singles.tile([P128, (T // P128) * DV], bf16, tag="Vsb")
            nc.gpsimd.memset(Vsb, 1.0)   # ones column survives V writes
            scr = singles.tile([1, 1], f32, tag="scr")
            nc.gpsimd.memset(scr, 0.0)
            nc.scalar.activation(scr, scr, ACTF.Exp, bias=0.0, scale=1.0)
            # keep PE continuously busy through the DMA-bound startup so the
            # p-state ramp reaches full clock before real work arrives
            warm = ps_m.tile([P128, 512], f32, tag="pm")
            for _ in range(64):
                nc.tensor.matmul(warm[:, :P128], identb, identb,
                                 start=True, stop=True)

            xts, xqs = {}, {}

            def load_kv(t8):
                xts[t8] = xin.tile([P128, 8, 512], f16, tag="xt", name=f"xt{t8}")
                nc.sync.dma_start(
                    out=xts[t8],
                    in_=xT[:, t8 * 512:(t8 + 1) * 512].rearrange(
                        "(c p) n -> p c n", p=P128))

            def load_q(i):
                # pair i's q columns: [256*i, 256*i+256)
                xqs[i] = qin.tile([P128, 8, 256], f16, tag="xq", name=f"xq{i}")
                nc.sync.dma_start(
                    out=xqs[i],
                    in_=xTq[:, i * 256:(i + 1) * 256].rearrange(
                        "(c p) n -> p c n", p=P128))

            def proj_kv(t8):
                xt = xts.pop(t8)
                kvps = ps_s.tile([P128, 512], f32, tag="ps")
                pairs = ([(c, c) for c in range(4)]
                         + [(c, c + 4) for c in range(4)]
                         + [(c + 4, c) for c in range(4)])
                for n, (wc, xc) in enumerate(pairs):
                    nc.tensor.matmul(kvps, wkv[:, wc, :], xt[:, xc, :],
                                     start=(n == 0), stop=(n == len(pairs) - 1))
                nc.scalar.activation(KH[:D, t8 * 512:(t8 + 1) * 512],
                                     kvps[:D, :], ACTF.Identity,
                                     bias=bks, scale=1.0)
                vtmp = vwork.tile([D, 512], bf16, tag="vtmp")
                nc.scalar.activation(vtmp, kvps[D:, :], ACTF.Identity,
                                     bias=bvs, scale=1.0)
                # transpose V^T [64, 128] blocks -> bf16 V-hat [128, 65] tiles
                for i in range(4):
                    t = t8 * 4 + i
                    pt = ps_m.tile([P128, 512], bf16, tag="pm")
                    nc.tensor.transpose(pt[:, :D],
                                        vtmp[:, i * P128:(i + 1) * P128],
                                        identb[:D, :D])
                    nc.vector.tensor_copy(Vsb[:, t * DV:t * DV + D], pt[:, :D])

            def proj_q(i):
                # pair i's Q: 256 columns
                xt = xqs.pop(i)
                qps = ps_s.tile([P128, 512], f32, tag="ps")
                pairs = ([(c, c) for c in range(4)]
                         + [(c, c + 4) for c in range(4)]
                         + [(c + 4, c) for c in range(4)])
                for n, (wc, xc) in enumerate(pairs):
                    nc.tensor.matmul(qps[:D, :256], wqs[:, wc, :], xt[:, xc, :],
                                     start=(n == 0), stop=(n == len(pairs) - 1))
                nc.scalar.activation(QH[:D, i * 256:(i + 1) * 256],
                                     qps[:D, :256], ACTF.Identity,
                                     bias=bqs, scale=1.0)

            def gen_pass1_pair(i):
                """Row max per slot of pair i; writes -m into QH.  Yields
                between emission units so pass-2 of the previous pair can be
                interleaved into the in-order PE queue (keeps PE busy while
                DVE drains pass-1 PSUM chunks; preserves the p-state ramp)."""
                for s in (2 * i, 2 * i + 1):
                    ncols = 256 * (s + 1)
                    chunks = [(off, min(512, ncols - off))
                              for off in range(0, ncols, 512)]
                    mgp = small.tile([P128, 8], f32, tag="mgp")
                    ci = 0
                    qsl = QH[:D, s * P128:(s + 1) * P128]
                    for (off, sw) in chunks[:-1]:
                        ps = ps_s.tile([P128, 512], f32, tag="ps")
                        nc.tensor.matmul(ps, qsl, KH[:D, off:off + sw],
                                         start=True, stop=True)
                        nc.vector.reduce_max(mgp[:, ci:ci + 1], ps, axis=AX)
                        ci += 1
                        yield
                    off, sw = chunks[-1]
                    ps = ps_s.tile([P128, 512], f32, tag="ps")
                    nc.tensor.matmul(ps[:, :sw], qsl, KH[:D, off:off + sw],
                                     start=True, stop=True)
                    nc.vector.tensor_add(ps[:, :sw], ps[:, :sw],
                                         msk[:, 512 - sw:])
                    nc.vector.reduce_max(mgp[:, ci:ci + 1], ps[:, :sw],
                                         axis=AX)
                    ci += 1
                    yield
                    mrunh = small.tile([P128, 1], bf16, tag="mrunh")
                    nc.vector.reduce_max(mrunh, mgp[:, :ci], axis=AX,
                                         negate=True)
                    # -m -> [1,128] row via PE transpose -> QH partition 64
                    pm = ps_m.tile([P128, 512], bf16, tag="pm")
                    nc.tensor.transpose(pm[:1, :P128], mrunh, identb)
                    nc.vector.tensor_copy(
                        QH[D:D + 1, s * P128:(s + 1) * P128], pm[:1, :P128])
                    yield

            def gen_pass2_pair(i):
                a, b = 2 * i, 2 * i + 1
                nfull = 4 * i + 2          # key blocks where both slots attend
                ecols = nfull * 256 + 256  # pass-2 staging cols incl. b tails
                while pend_finals:
                    pend_finals.pop(0)()
                AT = atp.tile([P128, 8192], bf16, tag="AT")
                qpr = QH[:, a * P128:a * P128 + 256]
                po = ps_o.tile([P128, 512], f32, tag="po", name="po_a")
                pob = ps_o.tile([P128, 512], f32, tag="po", name="po_b")
                av_a = av_b = 0

                last_goff = max(0, (ecols - 1) // 512 * 512 - 512)
                goff = 0
                while goff < ecols:
                    gw = min(512, ecols - goff)
                    ps2 = ps_e.tile([P128, 512], f32, tag="ps2")
                    seg = goff
                    while seg < goff + gw:
                        kb = seg // 256
                        if kb < nfull:
                            nc.tensor.matmul(
                                ps2[:, seg - goff:seg - goff + 256],
                                KH[:, kb * P128:(kb + 1) * P128], qpr,
                                start=True, stop=True)
                        else:
                            # two b-only tail blocks, 128 cols each
                            for tix in range(2):
                                kb2 = nfull + tix
                                so = seg - goff + tix * P128
                                nc.tensor.matmul(
                                    ps2[:, so:so + P128],
                                    KH[:, kb2 * P128:(kb2 + 1) * P128],
                                    QH[:, b * P128:(b + 1) * P128],
                                    start=True, stop=True)
                        seg += 256
                        yield
                    # causal masks on PSUM before exp
                    lo, hi = goff, goff + gw
                    m0 = (4 * i) * 256          # a-half of block 4i
                    if lo <= m0 < hi:
                        nc.vector.tensor_add(
                            ps2[:, m0 - goff:m0 - goff + P128],
                            ps2[:, m0 - goff:m0 - goff + P128],
                            mskT[:, 0:P128])
                    m1 = (4 * i + 1) * 256      # a-half of block 4i+1
                    if lo <= m1 < hi:
                        nc.vector.tensor_add(
                            ps2[:, m1 - goff:m1 - goff + P128],
                            ps2[:, m1 - goff:m1 - goff + P128],
                            mskT[:, P128:256])
                    mt = nfull * 256            # b tails
                    if lo <= mt < hi:
                        nc.vector.tensor_add(
                            ps2[:, mt - goff:mt - goff + 256],
                            ps2[:, mt - goff:mt - goff + 256],
                            mskT)
                    nc.scalar.activation(AT[:, goff:goff + gw], ps2[:, :gw],
                                         ACTF.Exp, bias=0.0, scale=8.0)
                    yield
                    # AV incrementally for both slots (separate PSUM banks);
                    # the FINAL group's AVs are deferred into the next pair's
                    # stream so the end-of-pair exp latency is covered
                    if goff < last_goff:
                        done = goff + gw
                        while (av_a + 1) * 256 <= done and av_a < nfull:
                            blk = av_a
                            nc.tensor.matmul(
                                po[:, :DV], AT[:, blk * 256:blk * 256 + P128],
                                Vsb[:, blk * DV:(blk + 1) * DV],
                                start=(blk == 0), stop=(blk == nfull - 1))
                            av_a += 1
                        while av_b < nfull and (av_b + 1) * 256 <= done:
                            blk = av_b
                            st = AT[:, blk * 256 + P128:(blk + 1) * 256]
                            nc.tensor.matmul(
                                pob[:, :DV], st,
                                Vsb[:, blk * DV:(blk + 1) * DV],
                                start=(blk == 0), stop=(blk == nfull + 1))
                            av_b += 1
                    yield
                    goff += gw

                def tail(a0=av_a, b0=av_b):
                    blk = a0
                    while blk < nfull:
                        nc.tensor.matmul(
                            po[:, :DV], AT[:, blk * 256:blk * 256 + P128],
                            Vsb[:, blk * DV:(blk + 1) * DV],
                            start=(blk == 0), stop=(blk == nfull - 1))
                        blk += 1
                    blk = b0
                    while blk < nfull + 2:
                        if blk < nfull:
                            st = AT[:, blk * 256 + P128:(blk + 1) * 256]
                        else:
                            st = AT[:, nfull * 256 + (blk - nfull) * P128:
                                    nfull * 256 + (blk - nfull + 1) * P128]
                        nc.tensor.matmul(
                            pob[:, :DV], st,
                            Vsb[:, blk * DV:(blk + 1) * DV],
                            start=(blk == 0), stop=(blk == nfull + 1))
                        blk += 1
                    for s, pot in ((a, po), (b, pob)):
                        rl = small.tile([P128, 1], f32, tag="rl")
                        nc.vector.reciprocal(rl, pot[:, D:D + 1])
                        yt = small.tile([P128, D], f32, tag="yt")
                        nc.vector.tensor_scalar_mul(yt, pot[:, :D], rl)
                        nc.sync.dma_start(out=y[s * P128:(s + 1) * P128, :],
                                          in_=yt)
                pend_finals.append(tail)
                yield

            def drain(gen):
                for _ in gen:
                    pass

            def interleave(g1, g2):
                """Pump g1 and g2 alternately (g2 twice per g1 step)."""
                done1 = done2 = False
                while not (done1 and done2):
                    if not done1:
                        done1 = next(g1, _SENTINEL) is _SENTINEL
                    if not done2:
                        for _ in range(3):
                            done2 = next(g2, _SENTINEL) is _SENTINEL
                            if done2:
                                break

            _SENTINEL = object()

            # ---- software-pipelined schedule ----
            load_kv(0)
            load_q(0)
            nc.sync.dma_start(out=msk, in_=maskp)
            load_kv(1)
            load_q(1)
            nc.sync.dma_start(out=mskT, in_=maskpT)
            nc.sync.dma_start(out=KH[D:D + 1, :], in_=onesr)
            prev = None
            pend_finals = []
            for i in range(NPAIR):
                if i + 2 < NPAIR:
                    load_kv(i + 2)
                if i + 2 < NPAIR:
                    load_q(i + 2)
                proj_kv(i)
                proj_q(i)
                g1 = gen_pass1_pair(i)
                if prev is None:
                    drain(g1)
                else:
                    interleave(g1, prev)
                prev = gen_pass2_pair(i)
            drain(prev)
            while pend_finals:
                pend_finals.pop(0)()

    nc.compile()
    return nc


def _get_nc():
    if "nc" not in _CACHED:
        _CACHED["nc"] = _build()
    return _CACHED["nc"]


def _prep_in_maps(x, Wq, bq, Wk, bk, Wv, bv):
    x = np.asarray(x, dtype=np.float32)
    Wq = np.asarray(Wq, dtype=np.float32)
    Wk = np.asarray(Wk, dtype=np.float32)
    Wv = np.asarray(Wv, dtype=np.float32)
    bq_ = np.asarray(bq, dtype=np.float32).reshape(D, 1)
    bk_ = np.asarray(bk, dtype=np.float32).reshape(D, 1)
    bv_ = np.asarray(bv, dtype=np.float32).reshape(D, 1)

    tri = np.triu(np.ones((P128, P128), np.float32), k=1) * NEG
    masks = []
    for P in range(2):
        mp = np.zeros((P128, 512), np.float32)
        if P == 0:
            mp[:, 256:384] = tri
            mp[:, 384:512] = NEG
        else:
            mp[:, 384:512] = tri
        masks.append(mp)

    masksT = []
    for P in range(2):
        mt = np.zeros((P128, 256), np.float32)
        mt[:, 0:128] = masks[P][:, 256:384].T
        mt[:, 128:256] = masks[P][:, 384:512].T
        masksT.append(mt)

    def hilo(a):
        """Stack [M, N] fp32 -> [2M, N] fp16 (hi rows, then residual rows)."""
        hi = a.astype(np.float16)
        lo = (a - hi.astype(np.float32)).astype(np.float16)
        return np.concatenate([hi, lo], axis=0)

    ones_row = np.ones((1, T), np.float32)
    wq2 = hilo(Wq)
    wkv2 = hilo(np.concatenate([Wk, Wv], axis=1))
    xTs = {}
    xTqs = {}
    in_maps = []
    for c in range(8):
        b, P = c // 2, c % 2
        if b not in xTs:
            xTs[b] = hilo(np.ascontiguousarray(x[b].T))
        if (b, P) not in xTqs:
            rows = ((np.arange(NSLOT) * 2 + P)[:, None] * P128
                    + np.arange(P128)[None, :]).reshape(-1)
            xTqs[(b, P)] = hilo(np.ascontiguousarray(x[b][rows].T))
        in_maps.append({
            "xT": xTs[b],
            "xTq": xTqs[(b, P)],
            "wqh": wq2, "wkv2": wkv2,
            "bq": bq_, "bk": bk_, "bv": bv_,
            "maskp": masks[P], "maskpT": masksT[P],
            "onesr": ones_row,
        })
    return in_maps


def _unshard(res):
    out = np.empty((B, T, D), np.float32)
    for c in range(8):
        b, P = c // 2, c % 2
        yl = res.results[c]["y"]
        for j in range(NSLOT):
            k = 2 * j + P
            out[b, k * P128:(k + 1) * P128] = yl[j * P128:(j + 1) * P128]
    return out


def kernel(x, Wq, bq, Wk, bk, Wv, bv):
    from concourse.bass_utils import run_bass_kernel_spmd

    in_maps = _prep_in_maps(x, Wq, bq, Wk, bk, Wv, bv)
    res = run_bass_kernel_spmd(_get_nc(), in_maps, core_ids=list(range(8)))
    _CACHED["last_results"] = res
    return _unshard(res)


if __name__ == "__main__":
    rng = np.random.default_rng(0)
    x = rng.standard_normal((B, T, C), dtype=np.float32)
    s = 1.0 / np.sqrt(C)
    Wq = rng.standard_normal((C, D), dtype=np.float32) * s
    Wk = rng.standard_normal((C, D), dtype=np.float32) * s
    Wv = rng.standard_normal((C, D), dtype=np.float32) * s
    z = np.zeros(D, np.float32)
    print(kernel(x, Wq, z, Wk, z, Wv, z).shape)
